# revision 8
# baseline (speedup 1.0000x reference)
"""Trainium2 Bass kernel for a dense transformer block.

Data-parallel over batch B=8 across 8 NeuronCores (one batch element per
core, weights replicated, no collectives).

Per core (x_b is [T=1024, C=1024] fp32):
  h  = LN1(x);  per-head q,k,v = h @ Wq/Wk/Wv;  S = q k^T / 8 with the
  "staircase" mask == block-causal at 64 granularity;  out = softmax(S) v
  x2 = x + cat(out) @ Wo + bo;  y = x2 + relu(LN2(x2) @ W1 + b1) @ W2 + b2

v3 design notes:
  - all matmul operands bf16 (weights pre-cast host-side); PSUM fp32.
  - LN affines folded into Wq/Wk/Wv/W1 host-side (h' = (x-m)*rstd only);
    the resulting per-channel biases enter as per-partition adds on the
    q/k copies, a K=1 ones-row matmul for v/bo/b2, and b1' in the relu.
  - attention A@V with V stationary ([keys,64+ones]), exp(S^T) moving:
    channel-major out, no output transposes; softmax denominator from
    the ones column, reciprocal on ScalarE (raw InstActivation),
    partition_broadcast on gpsimd, divide-multiply on DVE.
  - LN transposes in bf16, 8 per PSUM bank, single strided DVE copy out.
  - weights host-packed so every DMA is >=2KB/partition contiguous.
  - warm-up matmuls flip the PE HAM clock gate while x streams in.
"""

import os

import numpy as np

import concourse.bass as bass
import concourse.mybir as mybir
import concourse.tile as tile
from concourse import bacc
from concourse.masks import make_identity
from concourse.bass_utils import run_bass_kernel_spmd

T, C, H, HS = 1024, 1024, 16, 64
NT = T // 128
NCH = C // 128
NPAIR = H // 2
FF = 4 * C
NG = FF // 128
EPS = 1e-5
F32 = mybir.dt.float32
BF16 = mybir.dt.bfloat16


def _act_recip(nc, out, in_):
    """Reciprocal on ScalarE (accuracy ~1e-3, fine vs the 2e-2 gate)."""
    eng = nc.scalar
    ins = [eng.lower_ap(in_)]
    for v in (0.0, 1.0, 0.0):   # bias, scale, alpha
        ins.append(mybir.ImmediateValue(dtype=mybir.dt.float32, value=v))
    return eng.add_instruction(
        mybir.InstActivation(
            name=eng.bass.get_next_instruction_name(),
            func=mybir.ActivationFunctionType.Reciprocal,
            ins=ins, outs=[eng.lower_ap(out)]))


def _ln_stats(nc, pool, x_ap, eps_tile):
    """mean/rstd of [128,1024] fp32 tile -> (rstd, negmr) for ACT normalize."""
    stats = pool.tile([128, 2, 6], F32, tag="ln_stats", name="ln_stats")
    mv = pool.tile([128, 2], F32, tag="ln_mv", name="ln_mv")
    xr = x_ap.rearrange("p (s f) -> p s f", s=2)
    for s in range(2):
        nc.vector.bn_stats(out=stats[:, s, :], in_=xr[:, s, :])
    nc.vector.bn_aggr(out=mv, in_=stats)
    rstd = pool.tile([128, 1], F32, tag="ln_rstd", name="ln_rstd")
    nc.scalar.activation(
        out=rstd, in_=mv[:, 1:2],
        func=mybir.ActivationFunctionType.Sqrt,
        bias=eps_tile, scale=1.0,
    )
    nc.vector.reciprocal(out=rstd, in_=rstd)
    negmr = pool.tile([128, 1], F32, tag="ln_negmr", name="ln_negmr")
    nc.vector.tensor_scalar(
        out=negmr, in0=mv[:, 0:1], scalar1=rstd, scalar2=-1.0,
        op0=mybir.AluOpType.mult, op1=mybir.AluOpType.mult)
    return rstd, negmr


def build_program():
    nc = bacc.Bacc("TRN2", target_bir_lowering=False, debug=False, num_devices=8)

    x_d = nc.dram_tensor("x", [T, C], F32, kind="ExternalInput").ap()
    wq_d = nc.dram_tensor("wq", [NPAIR, 128, C], BF16, kind="ExternalInput").ap()
    wk_d = nc.dram_tensor("wk", [NPAIR, 128, C], BF16, kind="ExternalInput").ap()
    wv_d = nc.dram_tensor("wv", [128, NCH * C // 128 * 64 * 2], BF16,
                          kind="ExternalInput").ap()   # [128, 8192] packed
    wo_d = nc.dram_tensor("wo", [C, C], BF16, kind="ExternalInput").ap()
    w1_d = nc.dram_tensor("w1", [NG, 128, C], BF16, kind="ExternalInput").ap()
    w2_d = nc.dram_tensor("w2", [FF, C], BF16, kind="ExternalInput").ap()
    bq_d = nc.dram_tensor("bq", [C], F32, kind="ExternalInput").ap()
    bk_d = nc.dram_tensor("bk", [C], F32, kind="ExternalInput").ap()
    bv_d = nc.dram_tensor("bv", [C], BF16, kind="ExternalInput").ap()
    bo_d = nc.dram_tensor("bo", [C], BF16, kind="ExternalInput").ap()
    b1_d = nc.dram_tensor("b1", [FF], F32, kind="ExternalInput").ap()
    b2_d = nc.dram_tensor("b2", [C], BF16, kind="ExternalInput").ap()
    y_d = nc.dram_tensor("y", [T, C], F32, kind="ExternalOutput").ap()

    reps = int(os.environ.get("KERNEL_REPS", "1"))
    with tile.TileContext(nc) as tc:
        for r in range(reps):
            _emit(nc, tc, x_d, wq_d, wk_d, wv_d, wo_d, w1_d, w2_d,
                  bq_d, bk_d, bv_d, bo_d, b1_d, b2_d, y_d,
                  warmup=(r == 0))
    nc.compile()
    return nc


def _emit(nc, tc, x_d, wq_d, wk_d, wv_d, wo_d, w1_d, w2_d,
          bq_d, bk_d, bv_d, bo_d, b1_d, b2_d, y_d, warmup=False):
    singles = tc.alloc_tile_pool(name="singles", bufs=1)
    identf = singles.tile([128, 128], F32, name="identf")
    make_identity(nc, identf)
    ident = singles.tile([128, 128], BF16, name="ident")
    nc.vector.tensor_copy(out=ident, in_=identf)
    eps_tile = singles.tile([128, 1], F32, name="eps")
    nc.vector.memset(eps_tile, EPS)
    ones_r = singles.tile([1, 128], BF16, name="ones_r")
    nc.vector.memset(ones_r, 1.0)
    b1_sb = singles.tile([128, NG], F32, name="b1_sb")
    nc.sync.dma_start(out=b1_sb, in_=b1_d.rearrange("(g p) -> p g", p=128))
    bq_sb = singles.tile([128, NPAIR], F32, name="bq_sb")
    nc.sync.dma_start(out=bq_sb, in_=bq_d.rearrange("(g p) -> p g", p=128))
    bk_sb = singles.tile([128, NPAIR], F32, name="bk_sb")
    nc.sync.dma_start(out=bk_sb, in_=bk_d.rearrange("(g p) -> p g", p=128))
    bv_sb = singles.tile([1, C], BF16, name="bv_sb")
    nc.sync.dma_start(out=bv_sb, in_=bv_d.unsqueeze(0))
    bo_sb = singles.tile([1, C], BF16, name="bo_sb")
    nc.sync.dma_start(out=bo_sb, in_=bo_d.unsqueeze(0))
    b2_sb = singles.tile([1, C], BF16, name="b2_sb")
    nc.sync.dma_start(out=b2_sb, in_=b2_d.unsqueeze(0))

    ln_pool = tc.alloc_tile_pool(name="ln", bufs=3)

    # One global PSUM pool; every tile is one 2KB bank slot.
    ps_pool = tc.alloc_tile_pool(name="ps", bufs=1, space="PSUM")

    def big():
        return ps_pool.tile([128, 512], F32, tag="big", bufs=2, name="ps_big")

    if warmup:
        junk = singles.tile([128, 512], BF16, name="junk")
        nc.vector.memset(junk, 0.0)
        for _ in range(24):
            ps = ps_pool.tile([128, 512], F32, tag="av", bufs=2, name="ps_warm")
            nc.tensor.matmul(ps, ident, junk, start=True, stop=True)

    hT_pool = tc.alloc_tile_pool(name="hTp", bufs=1)
    hT = hT_pool.tile([128, NCH, T], BF16, name="hT")
    x2_pool = tc.alloc_tile_pool(name="x2p", bufs=1)
    x2 = x2_pool.tile([128, NT, C], F32, name="x2")
    w1_pool = tc.alloc_tile_pool(name="w1p", bufs=4)
    wo_pool = tc.alloc_tile_pool(name="wop", bufs=1)
    wo_t = wo_pool.tile([128, NCH, C], BF16, name="wo_t")
    v_pool = tc.alloc_tile_pool(name="vAp", bufs=1)
    v_all = v_pool.tile([128, NT, H * 65], BF16, name="v_all")
    for hh in range(H):
        nc.gpsimd.memset(v_all[:, :, 65 * hh + 64:65 * hh + 65], 1.0)
    w_pool = tc.alloc_tile_pool(name="wqk", bufs=2)

    h2T_pool = tc.alloc_tile_pool(name="h2Tp", bufs=1, side="right")
    h2T = h2T_pool.tile([128, NCH, T], BF16, name="h2T")
    outT_pool = tc.alloc_tile_pool(name="outTp", bufs=1, side="right")
    outT = outT_pool.tile([128, NPAIR, T], BF16, name="outT")

    wqk_tiles = {}

    def load_pair(pp):
        wq_t = w_pool.tile([128, C], BF16, tag="wq", name="wq_t")
        nc.sync.dma_start(out=wq_t, in_=wq_d[pp])
        wk_t = w_pool.tile([128, C], BF16, tag="wk", name="wk_t")
        nc.sync.dma_start(out=wk_t, in_=wk_d[pp])
        wqk_tiles[pp] = (wq_t, wk_t)

    # ---- Phase 1: LN1 -> hT (channel-major) + V ----
    with tc.tile_pool(name="h", bufs=3) as h_pool, \
         tc.tile_pool(name="xin1", bufs=3) as x_pool, \
         tc.tile_pool(name="wvg", bufs=1) as wv_pool:
        wv_t = wv_pool.tile([128, NCH * 1024], BF16, name="wv_t")
        x_ts = []
        for i in range(2):
            x_t = x_pool.tile([128, C], F32, tag="x", name="x_t")
            nc.sync.dma_start(out=x_t, in_=x_d[i * 128:(i + 1) * 128, :])
            x_ts.append(x_t)
        for grp in range(2):
            nc.sync.dma_start(
                out=wv_t[:, grp * 4096:(grp + 1) * 4096],
                in_=wv_d[:, grp * 4096:(grp + 1) * 4096])
        load_pair(0)
        load_pair(1)
        for i in range(NT):
            if i < 2:
                x_t = x_ts[i]
            else:
                x_t = x_pool.tile([128, C], F32, tag="x", name="x_t")
                nc.sync.dma_start(out=x_t, in_=x_d[i * 128:(i + 1) * 128, :])
            rstd, negmr = _ln_stats(nc, ln_pool, x_t, eps_tile)
            h_t = h_pool.tile([128, C], BF16, tag="h", name="h_t")
            nc.scalar.activation(
                out=h_t, in_=x_t,
                func=mybir.ActivationFunctionType.Identity,
                bias=negmr, scale=rstd)
            ps_tr = ps_pool.tile([128, 1024], BF16, tag=f"sc{i % 2}",
                                 bufs=2, name="ps_tr")
            for j in range(NCH):
                nc.tensor.transpose(
                    ps_tr[:, j * 128:(j + 1) * 128],
                    h_t[:, j * 128:(j + 1) * 128], ident)
            nc.vector.tensor_copy(
                out=hT[:, :, i * 128:(i + 1) * 128],
                in_=ps_tr.rearrange("p (j t) -> p j t", j=NCH))
            for grp in range(2):
                ps_v = big()
                nc.tensor.matmul(
                    ps_v, ones_r, bv_sb[0:1, grp * 512:(grp + 1) * 512],
                    start=True, stop=False)
                for j in range(NCH):
                    nc.tensor.matmul(
                        ps_v, hT[:, j, i * 128:(i + 1) * 128],
                        wv_t[:, j * 1024 + grp * 512:j * 1024 + (grp + 1) * 512],
                        start=False, stop=(j == NCH - 1))
                for hh in range(8):
                    head = grp * 8 + hh
                    nc.vector.tensor_copy(
                        out=v_all[:, i, 65 * head:65 * head + 64],
                        in_=ps_v[:, hh * 64:(hh + 1) * 64])

    # hoist wo loads: DMA queue is idle during attention
    for ch in range(NCH):
        nc.sync.dma_start(
            out=wo_t[:, ch, :], in_=wo_d[ch * 128:(ch + 1) * 128, :])

    # ---- Phase 2: per head-pair QK + attention (V stationary) ----
    with tc.tile_pool(name="qk", bufs=2) as qk_pool, \
         tc.tile_pool(name="expS", bufs=20) as e_pool, \
         tc.tile_pool(name="rec", bufs=4) as r_pool, \
         tc.tile_pool(name="rbc", bufs=4) as rb_pool:
        for p in range(NPAIR):
            if p + 2 < NPAIR:
                load_pair(p + 2)
            wq_t, wk_t = wqk_tiles.pop(p)
            qT = qk_pool.tile([128, T], BF16, tag="qT", name="qT")
            kT = qk_pool.tile([128, T], BF16, tag="kT", name="kT")
            for dst, wt, bias in ((qT, wq_t, bq_sb), (kT, wk_t, bk_sb)):
                for half in range(2):
                    ps = big()
                    for j in range(NCH):
                        nc.tensor.matmul(
                            ps, wt[:, j * 128:(j + 1) * 128],
                            hT[:, j, half * 512:(half + 1) * 512],
                            start=(j == 0), stop=(j == NCH - 1))
                    nc.vector.tensor_scalar(
                        out=dst[:, half * 512:(half + 1) * 512], in0=ps,
                        scalar1=bias[:, p:p + 1], scalar2=None,
                        op0=mybir.AluOpType.add)

            for th in range(2):
                t0 = th * 512
                njt = (th + 1) * 4
                eS = [[None] * njt for _ in range(2)]
                for hh in range(2):
                    hsl = slice(hh * 64, (hh + 1) * 64)
                    for j in range(njt):
                        c0 = max(0, j * 128 - t0)
                        ps = ps_pool.tile([128, 512], F32, tag=f"sc{hh}",
                                          bufs=2, name="ps_sc")
                        nc.tensor.matmul(
                            ps[:, c0:512],
                            kT[hsl, j * 128:(j + 1) * 128],
                            qT[hsl, t0 + c0:t0 + 512],
                            start=True, stop=True,
                            tile_position=(hh * 64, 0))
                        et = e_pool.tile([128, 512], BF16, tag="e", name="eS_t")
                        nc.scalar.activation(
                            out=et[:, c0:512], in_=ps[:, c0:512],
                            func=mybir.ActivationFunctionType.Exp,
                            scale=float(HS) ** -0.5)
                        if j * 128 >= t0:
                            nc.vector.memset(et[64:128, c0:c0 + 64], 0.0)
                        eS[hh][j] = et
                for hh in range(2):
                    head = 2 * p + hh
                    ps_av = ps_pool.tile([128, 512], F32, tag="av", bufs=2,
                                         name="ps_av")
                    for j in range(njt):
                        c0 = max(0, j * 128 - t0)
                        nc.tensor.matmul(
                            ps_av[0:65, c0:512],
                            v_all[:, j, 65 * head:65 * head + 65],
                            eS[hh][j][:, c0:512],
                            start=(j == 0), stop=(j == njt - 1))
                    rc = r_pool.tile([128, 512], F32, tag="rc", name="rc")
                    _act_recip(nc, rc[0:1, :], ps_av[64:65, :])
                    rb = rb_pool.tile([128, 512], F32, tag="rb", name="rb")
                    nc.gpsimd.partition_broadcast(rb[0:64, :], rc[0:1, :])
                    nc.vector.tensor_tensor(
                        out=outT[64 * hh:64 * (hh + 1), p, t0:t0 + 512],
                        in0=ps_av[0:64, :], in1=rb[0:64, :],
                        op=mybir.AluOpType.mult)
    v_pool.release()

    # hoist the first FFN1 weight tiles
    w1_tiles = {}

    def load_w1(gg):
        w1_t = w1_pool.tile([128, C], BF16, tag="w1", name="w1_t")
        nc.sync.dma_start(out=w1_t, in_=w1_d[gg])
        w1_tiles[gg] = w1_t

    for gg in range(3):
        load_w1(gg)

    # ---- Phase 3+4: projection + residual (+bo) + LN2 -> h2T ----
    with tc.tile_pool(name="xin2", bufs=3) as x_pool:
        for i in range(NT):
            x_t = x_pool.tile([128, C], F32, tag="x", name="x_t2")
            nc.sync.dma_start(out=x_t, in_=x_d[i * 128:(i + 1) * 128, :])
            for half in range(2):
                ps = big()
                nc.tensor.matmul(
                    ps, ones_r, bo_sb[0:1, half * 512:(half + 1) * 512],
                    start=True, stop=False)
                for ch in range(NCH):
                    nc.tensor.matmul(
                        ps, outT[:, ch, i * 128:(i + 1) * 128],
                        wo_t[:, ch, half * 512:(half + 1) * 512],
                        start=False, stop=(ch == NCH - 1))
                hsl = slice(half * 512, (half + 1) * 512)
                nc.vector.tensor_add(out=x2[:, i, hsl], in0=ps, in1=x_t[:, hsl])
            rstd, negmr = _ln_stats(nc, ln_pool, x2[:, i, :], eps_tile)
            h_t = x_pool.tile([128, C], BF16, tag="h2", name="h2_t")
            nc.scalar.activation(
                out=h_t, in_=x2[:, i, :],
                func=mybir.ActivationFunctionType.Identity,
                bias=negmr, scale=rstd)
            ps_tr = ps_pool.tile([128, 1024], BF16, tag=f"sc{i % 2}",
                                 bufs=2, name="ps_tr2")
            for j in range(NCH):
                nc.tensor.transpose(
                    ps_tr[:, j * 128:(j + 1) * 128],
                    h_t[:, j * 128:(j + 1) * 128], ident)
            nc.vector.tensor_copy(
                out=h2T[:, :, i * 128:(i + 1) * 128],
                in_=ps_tr.rearrange("p (j t) -> p j t", j=NCH))
    outT_pool.release()
    wo_pool.release()

    # ---- Phase 5: FFN. W1 streamed once into full-T uT; W2 in 4 passes ----
    with tc.tile_pool(name="w2", bufs=8) as w2_pool, \
         tc.tile_pool(name="uTp", bufs=1) as uT_pool, \
         tc.tile_pool(name="yout", bufs=4) as out_pool:
        uT = uT_pool.tile([128, NG, T], BF16, name="uT")
        for g in range(NG):
            if g + 3 < NG:
                load_w1(g + 3)
            w1_t = w1_tiles.pop(g)
            for th in range(2):
                ps = big()
                for j in range(NCH):
                    nc.tensor.matmul(
                        ps, w1_t[:, j * 128:(j + 1) * 128],
                        h2T[:, j, th * 512:(th + 1) * 512],
                        start=(j == 0), stop=(j == NCH - 1))
                nc.scalar.activation(
                    out=uT[:, g, th * 512:(th + 1) * 512], in_=ps,
                    func=mybir.ActivationFunctionType.Relu,
                    bias=b1_sb[:, g:g + 1], scale=1.0)
        for th in range(2):
            for chh in range(2):
                hsl = slice(chh * 512, (chh + 1) * 512)
                ps_f = [ps_pool.tile([128, 512], F32, tag=f"sc{it % 2}",
                                     bufs=2, name=f"ps_f{it}")
                        for it in range(4)]
                for it in range(4):
                    nc.tensor.matmul(
                        ps_f[it], ones_r, b2_sb[0:1, hsl],
                        start=True, stop=False)
                for k in range(NG):
                    w2_t = w2_pool.tile([128, 512], BF16, tag="w2", name="w2_t")
                    nc.sync.dma_start(
                        out=w2_t, in_=w2_d[k * 128:(k + 1) * 128, hsl])
                    for it in range(4):
                        nc.tensor.matmul(
                            ps_f[it],
                            uT[:, k, th * 512 + it * 128:th * 512 + (it + 1) * 128],
                            w2_t,
                            start=False, stop=(k == NG - 1))
                for it in range(4):
                    gi = th * 4 + it
                    o_t = out_pool.tile([128, 512], F32, tag="y", name="y_t")
                    nc.vector.tensor_add(
                        out=o_t, in0=ps_f[it], in1=x2[:, gi, hsl])
                    nc.sync.dma_start(
                        out=y_d[gi * 128:(gi + 1) * 128, hsl], in_=o_t)
    h2T_pool.release()
    w_pool.release()
    w1_pool.release()
    x2_pool.release()
    hT_pool.release()
    ps_pool.release()
    ln_pool.release()
    singles.release()


_NC_CACHE = {}


def _get_program():
    if "nc" not in _NC_CACHE:
        _NC_CACHE["nc"] = build_program()
    return _NC_CACHE["nc"]


def _prep_inputs(x, Wq, Wk, Wv, Wo, bo, ln1_g, ln1_b, ln2_g, ln2_b, W1, b1, W2, b2):
    import ml_dtypes
    BF = ml_dtypes.bfloat16
    f = lambda a: np.ascontiguousarray(np.asarray(a, dtype=np.float32))
    bf = lambda a: np.ascontiguousarray(np.asarray(a, np.float32).astype(BF))
    Wq, Wk, Wv = (np.asarray(w, np.float32) for w in (Wq, Wk, Wv))
    g1, b1l = np.asarray(ln1_g, np.float32), np.asarray(ln1_b, np.float32)
    g2, b2l = np.asarray(ln2_g, np.float32), np.asarray(ln2_b, np.float32)
    # [H,C,HS] -> [C, H*HS] with LN1 affine folded into the weights
    wq2 = Wq.transpose(1, 0, 2).reshape(C, C)
    wk2 = Wk.transpose(1, 0, 2).reshape(C, C)
    wv2 = Wv.transpose(1, 0, 2).reshape(C, C)
    bq, bk, bv = b1l @ wq2, b1l @ wk2, b1l @ wv2
    wq2, wk2, wv2 = g1[:, None] * wq2, g1[:, None] * wk2, g1[:, None] * wv2
    W1 = np.asarray(W1, np.float32)
    b1p = np.asarray(b1, np.float32) + b2l @ W1
    w1s = g2[:, None] * W1
    pack_p = lambda w: w.reshape(NCH, 128, NPAIR, 128).transpose(
        2, 1, 0, 3).reshape(NPAIR, 128, C)
    w1_pk = w1s.reshape(NCH, 128, NG, 128).transpose(2, 1, 0, 3).reshape(NG, 128, C)
    wv_pk = wv2.reshape(NCH, 128, C).transpose(1, 0, 2).reshape(128, NCH * C)
    return {
        "wq": bf(pack_p(wq2)), "wk": bf(pack_p(wk2)), "wv": bf(wv_pk),
        "wo": bf(Wo), "w1": bf(w1_pk), "w2": bf(W2),
        "bq": f(bq), "bk": f(bk), "bv": bf(bv),
        "bo": bf(bo), "b1": f(b1p), "b2": bf(b2),
    }


def kernel(x, mask, Wq, Wk, Wv, Wo, bo, ln1_g, ln1_b, ln2_g, ln2_b, W1, b1, W2, b2):
    x = np.ascontiguousarray(np.asarray(x, dtype=np.float32))
    B = x.shape[0]
    common = _prep_inputs(x, Wq, Wk, Wv, Wo, bo, ln1_g, ln1_b,
                          ln2_g, ln2_b, W1, b1, W2, b2)
    nc = _get_program()
    in_maps = [dict(common, x=np.ascontiguousarray(x[b])) for b in range(B)]
    res = run_bass_kernel_spmd(nc, in_maps, list(range(B)))
    return np.stack([res.results[b]["y"] for b in range(B)], axis=0)


# revision 11
# speedup vs baseline: 490.8822x; 490.8822x over previous
"""Trainium2 Bass kernel for a dense transformer block.

Data-parallel over batch B=8 across 8 NeuronCores (one batch element per
core, weights replicated, no collectives).

Per core (x_b is [T=1024, C=1024] fp32):
  h  = LN1(x);  per-head q,k,v = h @ Wq/Wk/Wv;  S = q k^T / 8 with the
  "staircase" mask == block-causal at 64 granularity;  out = softmax(S) v
  x2 = x + cat(out) @ Wo + bo;  y = x2 + relu(LN2(x2) @ W1 + b1) @ W2 + b2

v3 design notes:
  - all matmul operands bf16 (weights pre-cast host-side); PSUM fp32.
  - LN affines folded into Wq/Wk/Wv/W1 host-side (h' = (x-m)*rstd only);
    the resulting per-channel biases enter as per-partition adds on the
    q/k copies, a K=1 ones-row matmul for v/bo/b2, and b1' in the relu.
  - attention A@V with V stationary ([keys,64+ones]), exp(S^T) moving:
    channel-major out, no output transposes; softmax denominator from
    the ones column, reciprocal on ScalarE (raw InstActivation),
    partition_broadcast on gpsimd, divide-multiply on DVE.
  - LN transposes in bf16, 8 per PSUM bank, single strided DVE copy out.
  - weights host-packed so every DMA is >=2KB/partition contiguous.
  - warm-up matmuls flip the PE HAM clock gate while x streams in.
"""

import os

import numpy as np

import concourse.bass as bass
import concourse.mybir as mybir
import concourse.tile as tile
from concourse import bacc
from concourse.masks import make_identity
from concourse.bass_utils import run_bass_kernel_spmd

T, C, H, HS = 1024, 1024, 16, 64
NT = T // 128
NCH = C // 128
NPAIR = H // 2
FF = 4 * C
NG = FF // 128
EPS = 1e-5
F32 = mybir.dt.float32
BF16 = mybir.dt.bfloat16


def _act_recip(nc, out, in_):
    """Reciprocal on ScalarE (accuracy ~1e-3, fine vs the 2e-2 gate)."""
    eng = nc.scalar
    ins = [eng.lower_ap(in_)]
    for v in (0.0, 1.0, 0.0):   # bias, scale, alpha
        ins.append(mybir.ImmediateValue(dtype=mybir.dt.float32, value=v))
    return eng.add_instruction(
        mybir.InstActivation(
            name=eng.bass.get_next_instruction_name(),
            func=mybir.ActivationFunctionType.Reciprocal,
            ins=ins, outs=[eng.lower_ap(out)]))


def _ln_stats(nc, pool, x_ap, eps_tile):
    """mean/rstd of [128,1024] fp32 tile -> (rstd, negmr) for ACT normalize."""
    stats = pool.tile([128, 2, 6], F32, tag="ln_stats", name="ln_stats")
    mv = pool.tile([128, 2], F32, tag="ln_mv", name="ln_mv")
    xr = x_ap.rearrange("p (s f) -> p s f", s=2)
    for s in range(2):
        nc.vector.bn_stats(out=stats[:, s, :], in_=xr[:, s, :])
    nc.vector.bn_aggr(out=mv, in_=stats)
    rstd = pool.tile([128, 1], F32, tag="ln_rstd", name="ln_rstd")
    nc.scalar.activation(
        out=rstd, in_=mv[:, 1:2],
        func=mybir.ActivationFunctionType.Sqrt,
        bias=eps_tile, scale=1.0,
    )
    nc.vector.reciprocal(out=rstd, in_=rstd)
    negmr = pool.tile([128, 1], F32, tag="ln_negmr", name="ln_negmr")
    nc.vector.tensor_scalar(
        out=negmr, in0=mv[:, 0:1], scalar1=rstd, scalar2=-1.0,
        op0=mybir.AluOpType.mult, op1=mybir.AluOpType.mult)
    return rstd, negmr


def build_program():
    nc = bacc.Bacc("TRN2", target_bir_lowering=False, debug=False, num_devices=8)

    x_d = nc.dram_tensor("x", [T, C], F32, kind="ExternalInput").ap()
    wq_d = nc.dram_tensor("wq", [NPAIR, 128, C], BF16, kind="ExternalInput").ap()
    wk_d = nc.dram_tensor("wk", [NPAIR, 128, C], BF16, kind="ExternalInput").ap()
    wv_d = nc.dram_tensor("wv", [128, NCH * C // 128 * 64 * 2], BF16,
                          kind="ExternalInput").ap()   # [128, 8192] packed
    wo_d = nc.dram_tensor("wo", [C, C], BF16, kind="ExternalInput").ap()
    w1_d = nc.dram_tensor("w1", [NG, 128, C], BF16, kind="ExternalInput").ap()
    w2_d = nc.dram_tensor("w2", [FF, C], BF16, kind="ExternalInput").ap()
    bq_d = nc.dram_tensor("bq", [C], F32, kind="ExternalInput").ap()
    bk_d = nc.dram_tensor("bk", [C], F32, kind="ExternalInput").ap()
    bv_d = nc.dram_tensor("bv", [C], BF16, kind="ExternalInput").ap()
    bo_d = nc.dram_tensor("bo", [C], BF16, kind="ExternalInput").ap()
    b1_d = nc.dram_tensor("b1", [FF], F32, kind="ExternalInput").ap()
    b2_d = nc.dram_tensor("b2", [C], BF16, kind="ExternalInput").ap()
    y_d = nc.dram_tensor("y", [T, C], F32, kind="ExternalOutput").ap()

    reps = int(os.environ.get("KERNEL_REPS", "1"))
    with tile.TileContext(nc) as tc:
        for r in range(reps):
            _emit(nc, tc, x_d, wq_d, wk_d, wv_d, wo_d, w1_d, w2_d,
                  bq_d, bk_d, bv_d, bo_d, b1_d, b2_d, y_d,
                  warmup=(r == 0))
    nc.compile()
    return nc


def _emit(nc, tc, x_d, wq_d, wk_d, wv_d, wo_d, w1_d, w2_d,
          bq_d, bk_d, bv_d, bo_d, b1_d, b2_d, y_d, warmup=False):
    singles = tc.alloc_tile_pool(name="singles", bufs=1)
    identf = singles.tile([128, 128], F32, name="identf")
    make_identity(nc, identf)
    ident = singles.tile([128, 128], BF16, name="ident")
    nc.vector.tensor_copy(out=ident, in_=identf)
    eps_tile = singles.tile([128, 1], F32, name="eps")
    nc.vector.memset(eps_tile, EPS)
    ones_r = singles.tile([1, 128], BF16, name="ones_r")
    nc.vector.memset(ones_r, 1.0)
    b1_sb = singles.tile([128, NG], F32, name="b1_sb")
    nc.sync.dma_start(out=b1_sb, in_=b1_d.rearrange("(g p) -> p g", p=128))
    bq_sb = singles.tile([128, NPAIR], F32, name="bq_sb")
    nc.sync.dma_start(out=bq_sb, in_=bq_d.rearrange("(g p) -> p g", p=128))
    bk_sb = singles.tile([128, NPAIR], F32, name="bk_sb")
    nc.sync.dma_start(out=bk_sb, in_=bk_d.rearrange("(g p) -> p g", p=128))
    bv_sb = singles.tile([1, C], BF16, name="bv_sb")
    nc.sync.dma_start(out=bv_sb, in_=bv_d.unsqueeze(0))
    bo_sb = singles.tile([1, C], BF16, name="bo_sb")
    nc.sync.dma_start(out=bo_sb, in_=bo_d.unsqueeze(0))
    b2_sb = singles.tile([1, C], BF16, name="b2_sb")
    nc.sync.dma_start(out=b2_sb, in_=b2_d.unsqueeze(0))

    ln_pool = tc.alloc_tile_pool(name="ln", bufs=3)

    # One global PSUM pool; every tile is one 2KB bank slot.
    ps_pool = tc.alloc_tile_pool(name="ps", bufs=1, space="PSUM")

    def big():
        return ps_pool.tile([128, 512], F32, tag="big", bufs=2, name="ps_big")

    if warmup:
        junk = singles.tile([128, 512], BF16, name="junk")
        nc.vector.memset(junk, 0.0)
        for _ in range(24):
            ps = ps_pool.tile([128, 512], F32, tag="av", bufs=2, name="ps_warm")
            nc.tensor.matmul(ps, ident, junk, start=True, stop=True)

    hT_pool = tc.alloc_tile_pool(name="hTp", bufs=1)
    hT = hT_pool.tile([128, NCH, T], BF16, name="hT")
    x2_pool = tc.alloc_tile_pool(name="x2p", bufs=1)
    x2 = x2_pool.tile([128, NT, C], F32, name="x2")
    w1_pool = tc.alloc_tile_pool(name="w1p", bufs=4)
    wo_pool = tc.alloc_tile_pool(name="wop", bufs=1)
    wo_t = wo_pool.tile([128, NCH, C], BF16, name="wo_t")
    w_pool = tc.alloc_tile_pool(name="wqk", bufs=2)
    v_pool = tc.alloc_tile_pool(name="vAp", bufs=1)
    v_all = v_pool.tile([128, NT, H * 65], BF16, name="v_all")
    for hh in range(H):
        nc.gpsimd.memset(v_all[:, :, 65 * hh + 64:65 * hh + 65], 1.0)

    h2T_pool = tc.alloc_tile_pool(name="h2Tp", bufs=1, side="right")
    h2T = h2T_pool.tile([128, NCH, T], BF16, name="h2T")
    outT_pool = tc.alloc_tile_pool(name="outTp", bufs=1, side="right")
    outT = outT_pool.tile([128, NPAIR, T], BF16, name="outT")

    wqk_tiles = {}

    def load_pair(pp):
        wq_t = w_pool.tile([128, C], BF16, tag="wq", name="wq_t")
        nc.sync.dma_start(out=wq_t, in_=wq_d[pp])
        wk_t = w_pool.tile([128, C], BF16, tag="wk", name="wk_t")
        nc.sync.dma_start(out=wk_t, in_=wk_d[pp])
        wqk_tiles[pp] = (wq_t, wk_t)

    # ---- Phase 1: LN1 -> hT (channel-major) + V ----
    with tc.tile_pool(name="h", bufs=3) as h_pool, \
         tc.tile_pool(name="xin1", bufs=3) as x_pool, \
         tc.tile_pool(name="wvg", bufs=1) as wv_pool:
        wv_t = wv_pool.tile([128, NCH * 1024], BF16, name="wv_t")
        x_ts = []
        for i in range(2):
            x_t = x_pool.tile([128, C], F32, tag="x", name="x_t")
            nc.sync.dma_start(out=x_t, in_=x_d[i * 128:(i + 1) * 128, :])
            x_ts.append(x_t)
        for grp in range(2):
            nc.sync.dma_start(
                out=wv_t[:, grp * 4096:(grp + 1) * 4096],
                in_=wv_d[:, grp * 4096:(grp + 1) * 4096])
        load_pair(0)
        load_pair(1)
        for i in range(NT):
            if i < 2:
                x_t = x_ts[i]
            else:
                x_t = x_pool.tile([128, C], F32, tag="x", name="x_t")
                nc.sync.dma_start(out=x_t, in_=x_d[i * 128:(i + 1) * 128, :])
            rstd, negmr = _ln_stats(nc, ln_pool, x_t, eps_tile)
            h_t = h_pool.tile([128, C], BF16, tag="h", name="h_t")
            nc.scalar.activation(
                out=h_t, in_=x_t,
                func=mybir.ActivationFunctionType.Identity,
                bias=negmr, scale=rstd)
            ps_tr = ps_pool.tile([128, 1024], BF16, tag=f"sc{i % 2}",
                                 bufs=2, name="ps_tr")
            for j in range(NCH):
                nc.tensor.transpose(
                    ps_tr[:, j * 128:(j + 1) * 128],
                    h_t[:, j * 128:(j + 1) * 128], ident)
            nc.vector.tensor_copy(
                out=hT[:, :, i * 128:(i + 1) * 128],
                in_=ps_tr.rearrange("p (j t) -> p j t", j=NCH))
            for grp in range(2):
                ps_v = big()
                nc.tensor.matmul(
                    ps_v, ones_r, bv_sb[0:1, grp * 512:(grp + 1) * 512],
                    start=True, stop=False)
                for j in range(NCH):
                    nc.tensor.matmul(
                        ps_v, hT[:, j, i * 128:(i + 1) * 128],
                        wv_t[:, j * 1024 + grp * 512:j * 1024 + (grp + 1) * 512],
                        start=False, stop=(j == NCH - 1))
                for hh in range(8):
                    head = grp * 8 + hh
                    nc.vector.tensor_copy(
                        out=v_all[:, i, 65 * head:65 * head + 64],
                        in_=ps_v[:, hh * 64:(hh + 1) * 64])

    # hoist wo loads: DMA queue is idle during attention
    for ch in range(NCH):
        nc.sync.dma_start(
            out=wo_t[:, ch, :], in_=wo_d[ch * 128:(ch + 1) * 128, :])

    # ---- Phase 2: per head-pair QK + attention (V stationary) ----
    with tc.tile_pool(name="qk", bufs=2) as qk_pool, \
         tc.tile_pool(name="expS", bufs=20) as e_pool, \
         tc.tile_pool(name="rec", bufs=4) as r_pool, \
         tc.tile_pool(name="rbc", bufs=4) as rb_pool:
        for p in range(NPAIR):
            if p + 2 < NPAIR:
                load_pair(p + 2)
            wq_t, wk_t = wqk_tiles.pop(p)
            qT = qk_pool.tile([128, T], BF16, tag="qT", name="qT")
            kT = qk_pool.tile([128, T], BF16, tag="kT", name="kT")
            for dst, wt, bias in ((qT, wq_t, bq_sb), (kT, wk_t, bk_sb)):
                for half in range(2):
                    ps = big()
                    for j in range(NCH):
                        nc.tensor.matmul(
                            ps, wt[:, j * 128:(j + 1) * 128],
                            hT[:, j, half * 512:(half + 1) * 512],
                            start=(j == 0), stop=(j == NCH - 1))
                    nc.vector.tensor_scalar(
                        out=dst[:, half * 512:(half + 1) * 512], in0=ps,
                        scalar1=bias[:, p:p + 1], scalar2=None,
                        op0=mybir.AluOpType.add)

            for th in range(2):
                t0 = th * 512
                njt = (th + 1) * 4
                eS = [[None] * njt for _ in range(2)]
                for hh in range(2):
                    hsl = slice(hh * 64, (hh + 1) * 64)
                    for j in range(njt):
                        c0 = max(0, j * 128 - t0)
                        ps = ps_pool.tile([128, 512], F32, tag=f"sc{hh}",
                                          bufs=2, name="ps_sc")
                        nc.tensor.matmul(
                            ps[:, c0:512],
                            kT[hsl, j * 128:(j + 1) * 128],
                            qT[hsl, t0 + c0:t0 + 512],
                            start=True, stop=True,
                            tile_position=(hh * 64, 0))
                        et = e_pool.tile([128, 512], BF16, tag="e", name="eS_t")
                        nc.scalar.activation(
                            out=et[:, c0:512], in_=ps[:, c0:512],
                            func=mybir.ActivationFunctionType.Exp,
                            scale=float(HS) ** -0.5)
                        if j * 128 >= t0:
                            nc.vector.memset(et[64:128, c0:c0 + 64], 0.0)
                        eS[hh][j] = et
                for hh in range(2):
                    head = 2 * p + hh
                    ps_av = ps_pool.tile([128, 512], F32, tag="av", bufs=2,
                                         name="ps_av")
                    for j in range(njt):
                        c0 = max(0, j * 128 - t0)
                        nc.tensor.matmul(
                            ps_av[0:65, c0:512],
                            v_all[:, j, 65 * head:65 * head + 65],
                            eS[hh][j][:, c0:512],
                            start=(j == 0), stop=(j == njt - 1))
                    rc = r_pool.tile([128, 512], F32, tag="rc", name="rc")
                    _act_recip(nc, rc[0:1, :], ps_av[64:65, :])
                    rb = rb_pool.tile([128, 512], F32, tag="rb", name="rb")
                    nc.gpsimd.partition_broadcast(rb[0:64, :], rc[0:1, :])
                    nc.vector.tensor_tensor(
                        out=outT[64 * hh:64 * (hh + 1), p, t0:t0 + 512],
                        in0=ps_av[0:64, :], in1=rb[0:64, :],
                        op=mybir.AluOpType.mult)
    v_pool.release()
    w_pool.release()

    # hoist the first FFN1 weight tiles
    w1_tiles = {}

    def load_w1(gg):
        w1_t = w1_pool.tile([128, C], BF16, tag="w1", name="w1_t")
        nc.sync.dma_start(out=w1_t, in_=w1_d[gg])
        w1_tiles[gg] = w1_t

    for gg in range(3):
        load_w1(gg)

    # ---- Phase 3+4: projection + residual (+bo) + LN2 -> h2T ----
    with tc.tile_pool(name="xin2", bufs=3) as x_pool:
        for i in range(NT):
            x_t = x_pool.tile([128, C], F32, tag="x", name="x_t2")
            nc.sync.dma_start(out=x_t, in_=x_d[i * 128:(i + 1) * 128, :])
            for half in range(2):
                ps = big()
                nc.tensor.matmul(
                    ps, ones_r, bo_sb[0:1, half * 512:(half + 1) * 512],
                    start=True, stop=False)
                for ch in range(NCH):
                    nc.tensor.matmul(
                        ps, outT[:, ch, i * 128:(i + 1) * 128],
                        wo_t[:, ch, half * 512:(half + 1) * 512],
                        start=False, stop=(ch == NCH - 1))
                hsl = slice(half * 512, (half + 1) * 512)
                nc.vector.tensor_add(out=x2[:, i, hsl], in0=ps, in1=x_t[:, hsl])
            rstd, negmr = _ln_stats(nc, ln_pool, x2[:, i, :], eps_tile)
            h_t = x_pool.tile([128, C], BF16, tag="h2", name="h2_t")
            nc.scalar.activation(
                out=h_t, in_=x2[:, i, :],
                func=mybir.ActivationFunctionType.Identity,
                bias=negmr, scale=rstd)
            ps_tr = ps_pool.tile([128, 1024], BF16, tag=f"sc{i % 2}",
                                 bufs=2, name="ps_tr2")
            for j in range(NCH):
                nc.tensor.transpose(
                    ps_tr[:, j * 128:(j + 1) * 128],
                    h_t[:, j * 128:(j + 1) * 128], ident)
            nc.vector.tensor_copy(
                out=h2T[:, :, i * 128:(i + 1) * 128],
                in_=ps_tr.rearrange("p (j t) -> p j t", j=NCH))
    outT_pool.release()
    wo_pool.release()

    # ---- Phase 5: FFN. W1 streamed once into full-T uT; W2 in 4 passes ----
    with tc.tile_pool(name="w2", bufs=8) as w2_pool, \
         tc.tile_pool(name="uTp", bufs=1) as uT_pool, \
         tc.tile_pool(name="yout", bufs=4) as out_pool:
        uT = uT_pool.tile([128, NG, T], BF16, name="uT")
        for g in range(NG):
            if g + 3 < NG:
                load_w1(g + 3)
            w1_t = w1_tiles.pop(g)
            for th in range(2):
                ps = big()
                for j in range(NCH):
                    nc.tensor.matmul(
                        ps, w1_t[:, j * 128:(j + 1) * 128],
                        h2T[:, j, th * 512:(th + 1) * 512],
                        start=(j == 0), stop=(j == NCH - 1))
                nc.scalar.activation(
                    out=uT[:, g, th * 512:(th + 1) * 512], in_=ps,
                    func=mybir.ActivationFunctionType.Relu,
                    bias=b1_sb[:, g:g + 1], scale=1.0)
        for th in range(2):
            for chh in range(2):
                hsl = slice(chh * 512, (chh + 1) * 512)
                ps_f = [ps_pool.tile([128, 512], F32, tag=f"sc{it % 2}",
                                     bufs=2, name=f"ps_f{it}")
                        for it in range(4)]
                for it in range(4):
                    nc.tensor.matmul(
                        ps_f[it], ones_r, b2_sb[0:1, hsl],
                        start=True, stop=False)
                for k in range(NG):
                    w2_t = w2_pool.tile([128, 512], BF16, tag="w2", name="w2_t")
                    nc.sync.dma_start(
                        out=w2_t, in_=w2_d[k * 128:(k + 1) * 128, hsl])
                    for it in range(4):
                        nc.tensor.matmul(
                            ps_f[it],
                            uT[:, k, th * 512 + it * 128:th * 512 + (it + 1) * 128],
                            w2_t,
                            start=False, stop=(k == NG - 1))
                for it in range(4):
                    gi = th * 4 + it
                    o_t = out_pool.tile([128, 512], F32, tag="y", name="y_t")
                    nc.vector.tensor_add(
                        out=o_t, in0=ps_f[it], in1=x2[:, gi, hsl])
                    nc.sync.dma_start(
                        out=y_d[gi * 128:(gi + 1) * 128, hsl], in_=o_t)
    h2T_pool.release()
    w1_pool.release()
    x2_pool.release()
    hT_pool.release()
    ps_pool.release()
    ln_pool.release()
    singles.release()


_NC_CACHE = {}


def _get_program():
    if "nc" not in _NC_CACHE:
        _NC_CACHE["nc"] = build_program()
    return _NC_CACHE["nc"]


def _prep_inputs(x, Wq, Wk, Wv, Wo, bo, ln1_g, ln1_b, ln2_g, ln2_b, W1, b1, W2, b2):
    import ml_dtypes
    BF = ml_dtypes.bfloat16
    f = lambda a: np.ascontiguousarray(np.asarray(a, dtype=np.float32))
    bf = lambda a: np.ascontiguousarray(np.asarray(a, np.float32).astype(BF))
    Wq, Wk, Wv = (np.asarray(w, np.float32) for w in (Wq, Wk, Wv))
    g1, b1l = np.asarray(ln1_g, np.float32), np.asarray(ln1_b, np.float32)
    g2, b2l = np.asarray(ln2_g, np.float32), np.asarray(ln2_b, np.float32)
    # [H,C,HS] -> [C, H*HS] with LN1 affine folded into the weights
    wq2 = Wq.transpose(1, 0, 2).reshape(C, C)
    wk2 = Wk.transpose(1, 0, 2).reshape(C, C)
    wv2 = Wv.transpose(1, 0, 2).reshape(C, C)
    bq, bk, bv = b1l @ wq2, b1l @ wk2, b1l @ wv2
    wq2, wk2, wv2 = g1[:, None] * wq2, g1[:, None] * wk2, g1[:, None] * wv2
    W1 = np.asarray(W1, np.float32)
    b1p = np.asarray(b1, np.float32) + b2l @ W1
    w1s = g2[:, None] * W1
    pack_p = lambda w: w.reshape(NCH, 128, NPAIR, 128).transpose(
        2, 1, 0, 3).reshape(NPAIR, 128, C)
    w1_pk = w1s.reshape(NCH, 128, NG, 128).transpose(2, 1, 0, 3).reshape(NG, 128, C)
    wv_pk = wv2.reshape(NCH, 128, C).transpose(1, 0, 2).reshape(128, NCH * C)
    return {
        "wq": bf(pack_p(wq2)), "wk": bf(pack_p(wk2)), "wv": bf(wv_pk),
        "wo": bf(Wo), "w1": bf(w1_pk), "w2": bf(W2),
        "bq": f(bq), "bk": f(bk), "bv": bf(bv),
        "bo": bf(bo), "b1": f(b1p), "b2": bf(b2),
    }


def kernel(x, mask, Wq, Wk, Wv, Wo, bo, ln1_g, ln1_b, ln2_g, ln2_b, W1, b1, W2, b2):
    x = np.ascontiguousarray(np.asarray(x, dtype=np.float32))
    B = x.shape[0]
    common = _prep_inputs(x, Wq, Wk, Wv, Wo, bo, ln1_g, ln1_b,
                          ln2_g, ln2_b, W1, b1, W2, b2)
    nc = _get_program()
    in_maps = [dict(common, x=np.ascontiguousarray(x[b])) for b in range(B)]
    res = run_bass_kernel_spmd(nc, in_maps, list(range(B)))
    return np.stack([res.results[b]["y"] for b in range(B)], axis=0)


# revision 17
# speedup vs baseline: 919.6375x; 1.8734x over previous
"""Trainium2 Bass kernel for a dense transformer block.

Data-parallel over batch B=8 across 8 NeuronCores (one batch element per
core, weights replicated, no collectives).

Per core (x_b is [T=1024, C=1024] fp32):
  h  = LN1(x);  per-head q,k,v = h @ Wq/Wk/Wv;  S = q k^T / 8 with the
  "staircase" mask == block-causal at 64 granularity;  out = softmax(S) v
  x2 = x + cat(out) @ Wo + bo;  y = x2 + relu(LN2(x2) @ W1 + b1) @ W2 + b2

v3 design notes:
  - all matmul operands bf16 (weights pre-cast host-side); PSUM fp32.
  - LN affines folded into Wq/Wk/Wv/W1 host-side (h' = (x-m)*rstd only);
    the resulting per-channel biases enter as per-partition adds on the
    q/k copies, a K=1 ones-row matmul for v/bo/b2, and b1' in the relu.
  - attention A@V with V stationary ([keys,64+ones]), exp(S^T) moving:
    channel-major out, no output transposes; softmax denominator from
    the ones column, reciprocal on ScalarE (raw InstActivation),
    partition_broadcast on gpsimd, divide-multiply on DVE.
  - LN transposes in bf16, 8 per PSUM bank, single strided DVE copy out.
  - weights host-packed so every DMA is >=2KB/partition contiguous.
  - warm-up matmuls flip the PE HAM clock gate while x streams in.
"""

import os

import numpy as np

import concourse.bass as bass
import concourse.mybir as mybir
import concourse.tile as tile
from concourse import bacc
from concourse.masks import make_identity
from concourse.bass_utils import run_bass_kernel_spmd

T, C, H, HS = 1024, 1024, 16, 64
NT = T // 128
NCH = C // 128
NPAIR = H // 2
FF = 4 * C
NG = FF // 128
EPS = 1e-5
F32 = mybir.dt.float32
BF16 = mybir.dt.bfloat16


def _act_recip(nc, out, in_):
    """Reciprocal on ScalarE (accuracy ~1e-3, fine vs the 2e-2 gate)."""
    eng = nc.scalar
    ins = [eng.lower_ap(in_)]
    for v in (0.0, 1.0, 0.0):   # bias, scale, alpha
        ins.append(mybir.ImmediateValue(dtype=mybir.dt.float32, value=v))
    return eng.add_instruction(
        mybir.InstActivation(
            name=eng.bass.get_next_instruction_name(),
            func=mybir.ActivationFunctionType.Reciprocal,
            ins=ins, outs=[eng.lower_ap(out)]))


def _ln_stats(nc, pool, x_ap, eps_tile):
    """mean/rstd of [128,1024] fp32 tile -> (rstd, negmr) for ACT normalize."""
    stats = pool.tile([128, 2, 6], F32, tag="ln_stats", name="ln_stats")
    mv = pool.tile([128, 2], F32, tag="ln_mv", name="ln_mv")
    xr = x_ap.rearrange("p (s f) -> p s f", s=2)
    for s in range(2):
        nc.vector.bn_stats(out=stats[:, s, :], in_=xr[:, s, :])
    nc.vector.bn_aggr(out=mv, in_=stats)
    rstd = pool.tile([128, 1], F32, tag="ln_rstd", name="ln_rstd")
    nc.scalar.activation(
        out=rstd, in_=mv[:, 1:2],
        func=mybir.ActivationFunctionType.Sqrt,
        bias=eps_tile, scale=1.0,
    )
    nc.vector.reciprocal(out=rstd, in_=rstd)
    negmr = pool.tile([128, 1], F32, tag="ln_negmr", name="ln_negmr")
    nc.vector.tensor_scalar(
        out=negmr, in0=mv[:, 0:1], scalar1=rstd, scalar2=-1.0,
        op0=mybir.AluOpType.mult, op1=mybir.AluOpType.mult)
    return rstd, negmr


def build_program():
    nc = bacc.Bacc("TRN2", target_bir_lowering=False, debug=False, num_devices=8)

    x_d = nc.dram_tensor("x", [T, C], F32, kind="ExternalInput").ap()
    wq_d = nc.dram_tensor("wq", [NPAIR, 128, C], BF16, kind="ExternalInput").ap()
    wk_d = nc.dram_tensor("wk", [NPAIR, 128, C], BF16, kind="ExternalInput").ap()
    wv_d = nc.dram_tensor("wv", [128, NCH * C // 128 * 64 * 2], BF16,
                          kind="ExternalInput").ap()   # [128, 8192] packed
    wo_d = nc.dram_tensor("wo", [C, C], BF16, kind="ExternalInput").ap()
    w1_d = nc.dram_tensor("w1", [NG, 128, C], BF16, kind="ExternalInput").ap()
    w2_d = nc.dram_tensor("w2", [FF, C], BF16, kind="ExternalInput").ap()
    bq_d = nc.dram_tensor("bq", [C], F32, kind="ExternalInput").ap()
    bk_d = nc.dram_tensor("bk", [C], F32, kind="ExternalInput").ap()
    bv_d = nc.dram_tensor("bv", [C], BF16, kind="ExternalInput").ap()
    bo_d = nc.dram_tensor("bo", [C], BF16, kind="ExternalInput").ap()
    b1_d = nc.dram_tensor("b1", [FF], F32, kind="ExternalInput").ap()
    b2_d = nc.dram_tensor("b2", [C], BF16, kind="ExternalInput").ap()
    y_d = nc.dram_tensor("y", [T, C], F32, kind="ExternalOutput").ap()

    reps = int(os.environ.get("KERNEL_REPS", "1"))
    with tile.TileContext(nc) as tc:
        for r in range(reps):
            _emit(nc, tc, x_d, wq_d, wk_d, wv_d, wo_d, w1_d, w2_d,
                  bq_d, bk_d, bv_d, bo_d, b1_d, b2_d, y_d,
                  warmup=(r == 0))
    nc.compile()
    return nc


def _emit(nc, tc, x_d, wq_d, wk_d, wv_d, wo_d, w1_d, w2_d,
          bq_d, bk_d, bv_d, bo_d, b1_d, b2_d, y_d, warmup=False):
    singles = tc.alloc_tile_pool(name="singles", bufs=1)
    identf = singles.tile([128, 128], F32, name="identf")
    make_identity(nc, identf)
    ident = singles.tile([128, 128], BF16, name="ident")
    nc.vector.tensor_copy(out=ident, in_=identf)
    eps_tile = singles.tile([128, 1], F32, name="eps")
    nc.vector.memset(eps_tile, EPS)
    ones_r = singles.tile([1, 128], BF16, name="ones_r")
    nc.vector.memset(ones_r, 1.0)
    b1_sb = singles.tile([128, NG], F32, name="b1_sb")
    nc.sync.dma_start(out=b1_sb, in_=b1_d.rearrange("(g p) -> p g", p=128))
    bq_sb = singles.tile([128, NPAIR], F32, name="bq_sb")
    nc.sync.dma_start(out=bq_sb, in_=bq_d.rearrange("(g p) -> p g", p=128))
    bk_sb = singles.tile([128, NPAIR], F32, name="bk_sb")
    nc.sync.dma_start(out=bk_sb, in_=bk_d.rearrange("(g p) -> p g", p=128))
    bv_sb = singles.tile([1, C], BF16, name="bv_sb")
    nc.sync.dma_start(out=bv_sb, in_=bv_d.unsqueeze(0))
    bo_sb = singles.tile([1, C], BF16, name="bo_sb")
    nc.sync.dma_start(out=bo_sb, in_=bo_d.unsqueeze(0))
    b2_sb = singles.tile([1, C], BF16, name="b2_sb")
    nc.sync.dma_start(out=b2_sb, in_=b2_d.unsqueeze(0))

    ln_pool = tc.alloc_tile_pool(name="ln", bufs=3)

    # One global PSUM pool; every tile is one 2KB bank slot.
    ps_pool = tc.alloc_tile_pool(name="ps", bufs=1, space="PSUM")

    def big():
        return ps_pool.tile([128, 512], F32, tag="big", bufs=2, name="ps_big")

    if warmup:
        junk = singles.tile([128, 512], BF16, name="junk")
        nc.vector.memset(junk, 0.0)
        for _ in range(48):
            ps = ps_pool.tile([128, 512], F32, tag="av", bufs=2, name="ps_warm")
            nc.tensor.matmul(ps, ident, junk, start=True, stop=True)

    hT_pool = tc.alloc_tile_pool(name="hTp", bufs=1)
    hT = hT_pool.tile([128, NCH, T], BF16, name="hT")
    x2_pool = tc.alloc_tile_pool(name="x2p", bufs=1)
    x2 = x2_pool.tile([128, NT, C], F32, name="x2")
    w1_pool = tc.alloc_tile_pool(name="w1p", bufs=4)
    wo_pool = tc.alloc_tile_pool(name="wop", bufs=1)
    wo_t = wo_pool.tile([128, NCH, C], BF16, name="wo_t")
    w_pool = tc.alloc_tile_pool(name="wqk", bufs=2)
    v_pool = tc.alloc_tile_pool(name="vAp", bufs=1)
    v_all = v_pool.tile([128, NT, H * 65], BF16, name="v_all")
    for hh in range(H):
        nc.gpsimd.memset(v_all[:, :, 65 * hh + 64:65 * hh + 65], 1.0)

    h2T_pool = tc.alloc_tile_pool(name="h2Tp", bufs=1, side="right")
    h2T = h2T_pool.tile([128, NCH, T], BF16, name="h2T")
    outT_pool = tc.alloc_tile_pool(name="outTp", bufs=1, side="right")
    outT = outT_pool.tile([128, NPAIR, T], BF16, name="outT")

    wqk_tiles = {}

    def load_pair(pp):
        wq_t = w_pool.tile([128, C], BF16, tag="wq", name="wq_t")
        nc.sync.dma_start(out=wq_t, in_=wq_d[pp])
        wk_t = w_pool.tile([128, C], BF16, tag="wk", name="wk_t")
        nc.sync.dma_start(out=wk_t, in_=wk_d[pp])
        wqk_tiles[pp] = (wq_t, wk_t)

    # ---- Phase 1: LN1 -> hT (channel-major) + V ----
    v_view = v_all.rearrange("p i (h d) -> p i h d", h=H)
    with tc.tile_pool(name="h", bufs=3) as h_pool, \
         tc.tile_pool(name="xin1", bufs=4) as x_pool, \
         tc.tile_pool(name="wvg", bufs=1) as wv_pool:
        wv_t = wv_pool.tile([128, NCH * 1024], BF16, name="wv_t")

        def load_x(ii):
            x_t = x_pool.tile([128, C], F32, tag="x", name="x_t")
            for s in range(2):
                nc.sync.dma_start(
                    out=x_t[:, s * 512:(s + 1) * 512],
                    in_=x_d[ii * 128:(ii + 1) * 128, s * 512:(s + 1) * 512])
            return x_t

        x_ts = {0: load_x(0), 1: load_x(1)}
        for grp in range(2):
            nc.sync.dma_start(
                out=wv_t[:, grp * 4096:(grp + 1) * 4096],
                in_=wv_d[:, grp * 4096:(grp + 1) * 4096])
        load_pair(0)
        load_pair(1)
        stats = {0: _ln_stats(nc, ln_pool, x_ts[0], eps_tile)}
        for i in range(NT):
            if i + 2 < NT:
                x_ts[i + 2] = load_x(i + 2)
            rstd, negmr = stats.pop(i)
            x_t = x_ts.pop(i)
            h_t = h_pool.tile([128, C], BF16, tag="h", name="h_t")
            nc.scalar.activation(
                out=h_t, in_=x_t,
                func=mybir.ActivationFunctionType.Identity,
                bias=negmr, scale=rstd)
            ps_tr = ps_pool.tile([128, 1024], BF16, tag="sc01",
                                 bufs=2, name="ps_tr")
            for j in range(NCH):
                nc.tensor.transpose(
                    ps_tr[:, j * 128:(j + 1) * 128],
                    h_t[:, j * 128:(j + 1) * 128], ident)
            nc.vector.tensor_copy(
                out=hT[:, :, i * 128:(i + 1) * 128],
                in_=ps_tr.rearrange("p (j t) -> p j t", j=NCH))
            if i + 1 < NT:
                stats[i + 1] = _ln_stats(nc, ln_pool, x_ts[i + 1], eps_tile)
            for grp in range(2):
                ps_v = big()
                nc.tensor.matmul(
                    ps_v, ones_r, bv_sb[0:1, grp * 512:(grp + 1) * 512],
                    start=True, stop=False)
                for j in range(NCH):
                    nc.tensor.matmul(
                        ps_v, hT[:, j, i * 128:(i + 1) * 128],
                        wv_t[:, j * 1024 + grp * 512:j * 1024 + (grp + 1) * 512],
                        start=False, stop=(j == NCH - 1))
                nc.vector.tensor_copy(
                    out=v_view[:, i, grp * 8:(grp + 1) * 8, 0:64],
                    in_=ps_v.rearrange("p (h d) -> p h d", h=8))

    # hoist wo loads: DMA queue is idle during attention
    for ch in range(NCH):
        nc.sync.dma_start(
            out=wo_t[:, ch, :], in_=wo_d[ch * 128:(ch + 1) * 128, :])

    # ---- Phase 2: per head-pair QK + attention (V stationary) ----
    with tc.tile_pool(name="qk", bufs=2) as qk_pool, \
         tc.tile_pool(name="expS", bufs=20) as e_pool, \
         tc.tile_pool(name="rec", bufs=4) as r_pool, \
         tc.tile_pool(name="rbc", bufs=4) as rb_pool:
        for p in range(NPAIR):
            if p + 2 < NPAIR:
                load_pair(p + 2)
            wq_t, wk_t = wqk_tiles.pop(p)
            qT = qk_pool.tile([128, T], BF16, tag="qT", name="qT")
            kT = qk_pool.tile([128, T], BF16, tag="kT", name="kT")
            for dst, wt, bias in ((qT, wq_t, bq_sb), (kT, wk_t, bk_sb)):
                for half in range(2):
                    ps = big()
                    for j in range(NCH):
                        nc.tensor.matmul(
                            ps, wt[:, j * 128:(j + 1) * 128],
                            hT[:, j, half * 512:(half + 1) * 512],
                            start=(j == 0), stop=(j == NCH - 1))
                    nc.vector.tensor_scalar(
                        out=dst[:, half * 512:(half + 1) * 512], in0=ps,
                        scalar1=bias[:, p:p + 1], scalar2=None,
                        op0=mybir.AluOpType.add)

            for th in range(2):
                t0 = th * 512
                njt = (th + 1) * 4
                eS = [None] * njt
                for j in range(njt):
                    c0 = max(0, j * 128 - t0)
                    # both heads of the pair land in one 2-bank PSUM tile
                    ps = ps_pool.tile([128, 1024], F32, tag="sc01",
                                      bufs=2, name="ps_sc")
                    for hh in range(2):
                        hsl = slice(hh * 64, (hh + 1) * 64)
                        nc.tensor.matmul(
                            ps[:, hh * 512 + c0:(hh + 1) * 512],
                            kT[hsl, j * 128:(j + 1) * 128],
                            qT[hsl, t0 + c0:t0 + 512],
                            start=True, stop=True,
                            tile_position=(hh * 64, 0))
                    et = e_pool.tile([128, 1024], BF16, tag="e", name="eS_t")
                    pv = ps.rearrange("p (h q) -> p h q", h=2)
                    ev = et.rearrange("p (h q) -> p h q", h=2)
                    nc.scalar.activation(
                        out=ev[:, :, c0:512], in_=pv[:, :, c0:512],
                        func=mybir.ActivationFunctionType.Exp,
                        scale=float(HS) ** -0.5)
                    if j * 128 >= t0:
                        for hh in range(2):
                            nc.vector.memset(
                                et[64:128, hh * 512 + c0:hh * 512 + c0 + 64], 0.0)
                    eS[j] = et
                for hh in range(2):
                    head = 2 * p + hh
                    ps_av = ps_pool.tile([128, 512], F32, tag="av", bufs=2,
                                         name="ps_av")
                    for j in range(njt):
                        c0 = max(0, j * 128 - t0)
                        nc.tensor.matmul(
                            ps_av[0:65, c0:512],
                            v_all[:, j, 65 * head:65 * head + 65],
                            eS[j][:, hh * 512 + c0:(hh + 1) * 512],
                            start=(j == 0), stop=(j == njt - 1))
                    rc = r_pool.tile([128, 512], F32, tag="rc", name="rc")
                    _act_recip(nc, rc[0:1, :], ps_av[64:65, :])
                    rb = rb_pool.tile([128, 512], F32, tag="rb", name="rb")
                    nc.gpsimd.partition_broadcast(rb[0:64, :], rc[0:1, :])
                    nc.vector.tensor_tensor(
                        out=outT[64 * hh:64 * (hh + 1), p, t0:t0 + 512],
                        in0=ps_av[0:64, :], in1=rb[0:64, :],
                        op=mybir.AluOpType.mult)
    v_pool.release()
    w_pool.release()

    # hoist the first FFN1 weight tiles
    w1_tiles = {}

    def load_w1(gg):
        w1_t = w1_pool.tile([128, C], BF16, tag="w1", name="w1_t")
        nc.sync.dma_start(out=w1_t, in_=w1_d[gg])
        w1_tiles[gg] = w1_t

    for gg in range(3):
        load_w1(gg)

    # ---- Phase 3+4: projection + residual (+bo) + LN2 -> h2T ----
    # Skewed: transposes of tile i-1 are emitted after tile i's proj
    # matmuls so the PE never waits on the DVE/ACT LN2 chain.
    with tc.tile_pool(name="xin2", bufs=4) as x_pool:

        def load_x2(ii):
            x_t = x_pool.tile([128, C], F32, tag="x", name="x_t2")
            for s in range(2):
                nc.sync.dma_start(
                    out=x_t[:, s * 512:(s + 1) * 512],
                    in_=x_d[ii * 128:(ii + 1) * 128, s * 512:(s + 1) * 512])
            return x_t

        x_ts2 = {0: load_x2(0), 1: load_x2(1)}
        h_ts = {}

        def emit_tr2(ii):
            h_t = h_ts.pop(ii)
            ps_tr = ps_pool.tile([128, 1024], BF16, tag="sc01",
                                 bufs=2, name="ps_tr2")
            for j in range(NCH):
                nc.tensor.transpose(
                    ps_tr[:, j * 128:(j + 1) * 128],
                    h_t[:, j * 128:(j + 1) * 128], ident)
            nc.vector.tensor_copy(
                out=h2T[:, :, ii * 128:(ii + 1) * 128],
                in_=ps_tr.rearrange("p (j t) -> p j t", j=NCH))

        for i in range(NT):
            if i + 2 < NT:
                x_ts2[i + 2] = load_x2(i + 2)
            x_t = x_ts2.pop(i)
            for half in range(2):
                ps = big()
                nc.tensor.matmul(
                    ps, ones_r, bo_sb[0:1, half * 512:(half + 1) * 512],
                    start=True, stop=False)
                for ch in range(NCH):
                    nc.tensor.matmul(
                        ps, outT[:, ch, i * 128:(i + 1) * 128],
                        wo_t[:, ch, half * 512:(half + 1) * 512],
                        start=False, stop=(ch == NCH - 1))
                hsl = slice(half * 512, (half + 1) * 512)
                nc.vector.tensor_add(out=x2[:, i, hsl], in0=ps, in1=x_t[:, hsl])
            rstd, negmr = _ln_stats(nc, ln_pool, x2[:, i, :], eps_tile)
            h_t = x_pool.tile([128, C], BF16, tag="h2", name="h2_t")
            nc.scalar.activation(
                out=h_t, in_=x2[:, i, :],
                func=mybir.ActivationFunctionType.Identity,
                bias=negmr, scale=rstd)
            h_ts[i] = h_t
            if i >= 1:
                emit_tr2(i - 1)
        emit_tr2(NT - 1)
    outT_pool.release()
    wo_pool.release()

    # ---- Phase 5: FFN. W1 streamed once into full-T uT; W2 in 4 passes ----
    with tc.tile_pool(name="w2", bufs=8) as w2_pool, \
         tc.tile_pool(name="uTp", bufs=1) as uT_pool, \
         tc.tile_pool(name="yout", bufs=4) as out_pool:
        uT = uT_pool.tile([128, NG, T], BF16, name="uT")
        for g in range(NG):
            if g + 3 < NG:
                load_w1(g + 3)
            w1_t = w1_tiles.pop(g)
            for th in range(2):
                ps = big()
                for j in range(NCH):
                    nc.tensor.matmul(
                        ps, w1_t[:, j * 128:(j + 1) * 128],
                        h2T[:, j, th * 512:(th + 1) * 512],
                        start=(j == 0), stop=(j == NCH - 1))
                nc.vector.tensor_scalar(
                    out=uT[:, g, th * 512:(th + 1) * 512], in0=ps,
                    scalar1=b1_sb[:, g:g + 1], scalar2=0.0,
                    op0=mybir.AluOpType.add, op1=mybir.AluOpType.max)
        for th in range(2):
            for chh in range(2):
                hsl = slice(chh * 512, (chh + 1) * 512)
                ps_w = [ps_pool.tile([128, 1024], F32, tag="sc01",
                                     bufs=2, name=f"ps_w{iw}")
                        for iw in range(2)]
                ps_f = [ps_w[it // 2][:, (it % 2) * 512:(it % 2 + 1) * 512]
                        for it in range(4)]
                for it in range(4):
                    nc.tensor.matmul(
                        ps_f[it], ones_r, b2_sb[0:1, hsl],
                        start=True, stop=False)
                for k in range(NG):
                    w2_t = w2_pool.tile([128, 512], BF16, tag="w2", name="w2_t")
                    nc.sync.dma_start(
                        out=w2_t, in_=w2_d[k * 128:(k + 1) * 128, hsl])
                    for it in range(4):
                        nc.tensor.matmul(
                            ps_f[it],
                            uT[:, k, th * 512 + it * 128:th * 512 + (it + 1) * 128],
                            w2_t,
                            start=False, stop=(k == NG - 1))
                for it in range(4):
                    gi = th * 4 + it
                    o_t = out_pool.tile([128, 512], F32, tag="y", name="y_t")
                    nc.vector.tensor_add(
                        out=o_t, in0=ps_f[it], in1=x2[:, gi, hsl])
                    nc.sync.dma_start(
                        out=y_d[gi * 128:(gi + 1) * 128, hsl], in_=o_t)
    h2T_pool.release()
    w1_pool.release()
    x2_pool.release()
    hT_pool.release()
    ps_pool.release()
    ln_pool.release()
    singles.release()


_NC_CACHE = {}


def _get_program():
    if "nc" not in _NC_CACHE:
        _NC_CACHE["nc"] = build_program()
    return _NC_CACHE["nc"]


def _prep_inputs(x, Wq, Wk, Wv, Wo, bo, ln1_g, ln1_b, ln2_g, ln2_b, W1, b1, W2, b2):
    import ml_dtypes
    BF = ml_dtypes.bfloat16
    f = lambda a: np.ascontiguousarray(np.asarray(a, dtype=np.float32))
    bf = lambda a: np.ascontiguousarray(np.asarray(a, np.float32).astype(BF))
    Wq, Wk, Wv = (np.asarray(w, np.float32) for w in (Wq, Wk, Wv))
    g1, b1l = np.asarray(ln1_g, np.float32), np.asarray(ln1_b, np.float32)
    g2, b2l = np.asarray(ln2_g, np.float32), np.asarray(ln2_b, np.float32)
    # [H,C,HS] -> [C, H*HS] with LN1 affine folded into the weights
    wq2 = Wq.transpose(1, 0, 2).reshape(C, C)
    wk2 = Wk.transpose(1, 0, 2).reshape(C, C)
    wv2 = Wv.transpose(1, 0, 2).reshape(C, C)
    bq, bk, bv = b1l @ wq2, b1l @ wk2, b1l @ wv2
    wq2, wk2, wv2 = g1[:, None] * wq2, g1[:, None] * wk2, g1[:, None] * wv2
    W1 = np.asarray(W1, np.float32)
    b1p = np.asarray(b1, np.float32) + b2l @ W1
    w1s = g2[:, None] * W1
    pack_p = lambda w: w.reshape(NCH, 128, NPAIR, 128).transpose(
        2, 1, 0, 3).reshape(NPAIR, 128, C)
    w1_pk = w1s.reshape(NCH, 128, NG, 128).transpose(2, 1, 0, 3).reshape(NG, 128, C)
    wv_pk = wv2.reshape(NCH, 128, C).transpose(1, 0, 2).reshape(128, NCH * C)
    return {
        "wq": bf(pack_p(wq2)), "wk": bf(pack_p(wk2)), "wv": bf(wv_pk),
        "wo": bf(Wo), "w1": bf(w1_pk), "w2": bf(W2),
        "bq": f(bq), "bk": f(bk), "bv": bf(bv),
        "bo": bf(bo), "b1": f(b1p), "b2": bf(b2),
    }


def kernel(x, mask, Wq, Wk, Wv, Wo, bo, ln1_g, ln1_b, ln2_g, ln2_b, W1, b1, W2, b2):
    x = np.ascontiguousarray(np.asarray(x, dtype=np.float32))
    B = x.shape[0]
    common = _prep_inputs(x, Wq, Wk, Wv, Wo, bo, ln1_g, ln1_b,
                          ln2_g, ln2_b, W1, b1, W2, b2)
    nc = _get_program()
    in_maps = [dict(common, x=np.ascontiguousarray(x[b])) for b in range(B)]
    res = run_bass_kernel_spmd(nc, in_maps, list(range(B)))
    return np.stack([res.results[b]["y"] for b in range(B)], axis=0)


# revision 18
# speedup vs baseline: 946.4172x; 1.0291x over previous
"""Trainium2 Bass kernel for a dense transformer block.

Data-parallel over batch B=8 across 8 NeuronCores (one batch element per
core, weights replicated, no collectives).

Per core (x_b is [T=1024, C=1024] fp32):
  h  = LN1(x);  per-head q,k,v = h @ Wq/Wk/Wv;  S = q k^T / 8 with the
  "staircase" mask == block-causal at 64 granularity;  out = softmax(S) v
  x2 = x + cat(out) @ Wo + bo;  y = x2 + relu(LN2(x2) @ W1 + b1) @ W2 + b2

v3 design notes:
  - all matmul operands bf16 (weights pre-cast host-side); PSUM fp32.
  - LN affines folded into Wq/Wk/Wv/W1 host-side (h' = (x-m)*rstd only);
    the resulting per-channel biases enter as per-partition adds on the
    q/k copies, a K=1 ones-row matmul for v/bo/b2, and b1' in the relu.
  - attention A@V with V stationary ([keys,64+ones]), exp(S^T) moving:
    channel-major out, no output transposes; softmax denominator from
    the ones column, reciprocal on ScalarE (raw InstActivation),
    partition_broadcast on gpsimd, divide-multiply on DVE.
  - LN transposes in bf16, 8 per PSUM bank, single strided DVE copy out.
  - weights host-packed so every DMA is >=2KB/partition contiguous.
  - warm-up matmuls flip the PE HAM clock gate while x streams in.
"""

import os

import numpy as np

import concourse.bass as bass
import concourse.mybir as mybir
import concourse.tile as tile
from concourse import bacc
from concourse.masks import make_identity
from concourse.bass_utils import run_bass_kernel_spmd

T, C, H, HS = 1024, 1024, 16, 64
NT = T // 128
NCH = C // 128
NPAIR = H // 2
FF = 4 * C
NG = FF // 128
EPS = 1e-5
F32 = mybir.dt.float32
BF16 = mybir.dt.bfloat16


def _act_recip(nc, out, in_):
    """Reciprocal on ScalarE (accuracy ~1e-3, fine vs the 2e-2 gate)."""
    eng = nc.scalar
    ins = [eng.lower_ap(in_)]
    for v in (0.0, 1.0, 0.0):   # bias, scale, alpha
        ins.append(mybir.ImmediateValue(dtype=mybir.dt.float32, value=v))
    return eng.add_instruction(
        mybir.InstActivation(
            name=eng.bass.get_next_instruction_name(),
            func=mybir.ActivationFunctionType.Reciprocal,
            ins=ins, outs=[eng.lower_ap(out)]))


def _ln_stats(nc, pool, x_ap, eps_tile):
    """mean/rstd of [128,1024] fp32 tile -> (rstd, negmr) for ACT normalize."""
    stats = pool.tile([128, 2, 6], F32, tag="ln_stats", name="ln_stats")
    mv = pool.tile([128, 2], F32, tag="ln_mv", name="ln_mv")
    xr = x_ap.rearrange("p (s f) -> p s f", s=2)
    for s in range(2):
        nc.vector.bn_stats(out=stats[:, s, :], in_=xr[:, s, :])
    nc.vector.bn_aggr(out=mv, in_=stats)
    rstd = pool.tile([128, 1], F32, tag="ln_rstd", name="ln_rstd")
    nc.scalar.activation(
        out=rstd, in_=mv[:, 1:2],
        func=mybir.ActivationFunctionType.Sqrt,
        bias=eps_tile, scale=1.0,
    )
    nc.vector.reciprocal(out=rstd, in_=rstd)
    negmr = pool.tile([128, 1], F32, tag="ln_negmr", name="ln_negmr")
    nc.vector.tensor_scalar(
        out=negmr, in0=mv[:, 0:1], scalar1=rstd, scalar2=-1.0,
        op0=mybir.AluOpType.mult, op1=mybir.AluOpType.mult)
    return rstd, negmr


def build_program():
    nc = bacc.Bacc("TRN2", target_bir_lowering=False, debug=False, num_devices=8)

    x_d = nc.dram_tensor("x", [T, C], F32, kind="ExternalInput").ap()
    wq_d = nc.dram_tensor("wq", [NPAIR, 128, C], BF16, kind="ExternalInput").ap()
    wk_d = nc.dram_tensor("wk", [NPAIR, 128, C], BF16, kind="ExternalInput").ap()
    wv_d = nc.dram_tensor("wv", [128, NCH * C // 128 * 64 * 2], BF16,
                          kind="ExternalInput").ap()   # [128, 8192] packed
    wo_d = nc.dram_tensor("wo", [C, C], BF16, kind="ExternalInput").ap()
    w1_d = nc.dram_tensor("w1", [NG, 128, C], BF16, kind="ExternalInput").ap()
    w2_d = nc.dram_tensor("w2", [FF, C], BF16, kind="ExternalInput").ap()
    bq_d = nc.dram_tensor("bq", [C], F32, kind="ExternalInput").ap()
    bk_d = nc.dram_tensor("bk", [C], F32, kind="ExternalInput").ap()
    bv_d = nc.dram_tensor("bv", [C], BF16, kind="ExternalInput").ap()
    bo_d = nc.dram_tensor("bo", [C], BF16, kind="ExternalInput").ap()
    b1_d = nc.dram_tensor("b1", [FF], F32, kind="ExternalInput").ap()
    b2_d = nc.dram_tensor("b2", [C], BF16, kind="ExternalInput").ap()
    y_d = nc.dram_tensor("y", [T, C], F32, kind="ExternalOutput").ap()

    reps = int(os.environ.get("KERNEL_REPS", "1"))
    with tile.TileContext(nc) as tc:
        for r in range(reps):
            _emit(nc, tc, x_d, wq_d, wk_d, wv_d, wo_d, w1_d, w2_d,
                  bq_d, bk_d, bv_d, bo_d, b1_d, b2_d, y_d,
                  warmup=(r == 0))
    nc.compile()
    return nc


def _emit(nc, tc, x_d, wq_d, wk_d, wv_d, wo_d, w1_d, w2_d,
          bq_d, bk_d, bv_d, bo_d, b1_d, b2_d, y_d, warmup=False):
    singles = tc.alloc_tile_pool(name="singles", bufs=1)
    identf = singles.tile([128, 128], F32, name="identf")
    make_identity(nc, identf)
    ident = singles.tile([128, 128], BF16, name="ident")
    nc.vector.tensor_copy(out=ident, in_=identf)
    eps_tile = singles.tile([128, 1], F32, name="eps")
    nc.vector.memset(eps_tile, EPS)
    ones_r = singles.tile([1, 128], BF16, name="ones_r")
    nc.vector.memset(ones_r, 1.0)
    b1_sb = singles.tile([128, NG], F32, name="b1_sb")
    nc.sync.dma_start(out=b1_sb, in_=b1_d.rearrange("(g p) -> p g", p=128))
    bq_sb = singles.tile([128, NPAIR], F32, name="bq_sb")
    nc.sync.dma_start(out=bq_sb, in_=bq_d.rearrange("(g p) -> p g", p=128))
    bk_sb = singles.tile([128, NPAIR], F32, name="bk_sb")
    nc.sync.dma_start(out=bk_sb, in_=bk_d.rearrange("(g p) -> p g", p=128))
    bv_sb = singles.tile([1, C], BF16, name="bv_sb")
    nc.sync.dma_start(out=bv_sb, in_=bv_d.unsqueeze(0))
    bo_sb = singles.tile([1, C], BF16, name="bo_sb")
    nc.sync.dma_start(out=bo_sb, in_=bo_d.unsqueeze(0))
    b2_sb = singles.tile([1, C], BF16, name="b2_sb")
    nc.sync.dma_start(out=b2_sb, in_=b2_d.unsqueeze(0))

    ln_pool = tc.alloc_tile_pool(name="ln", bufs=3)

    # One global PSUM pool; every tile is one 2KB bank slot.
    ps_pool = tc.alloc_tile_pool(name="ps", bufs=1, space="PSUM")

    def big():
        return ps_pool.tile([128, 512], F32, tag="big", bufs=2, name="ps_big")

    if warmup:
        junk = singles.tile([128, 512], BF16, name="junk")
        nc.vector.memset(junk, 0.0)
        for _ in range(48):
            ps = ps_pool.tile([128, 512], F32, tag="av", bufs=2, name="ps_warm")
            nc.tensor.matmul(ps, ident, junk, start=True, stop=True)

    hT_pool = tc.alloc_tile_pool(name="hTp", bufs=1)
    hT = hT_pool.tile([128, NCH, T], BF16, name="hT")
    x2_pool = tc.alloc_tile_pool(name="x2p", bufs=1)
    x2 = x2_pool.tile([128, NT, C], F32, name="x2")
    w1_pool = tc.alloc_tile_pool(name="w1p", bufs=4)
    wo_pool = tc.alloc_tile_pool(name="wop", bufs=1)
    wo_t = wo_pool.tile([128, NCH, C], BF16, name="wo_t")
    w_pool = tc.alloc_tile_pool(name="wqk", bufs=2)
    v_pool = tc.alloc_tile_pool(name="vAp", bufs=1)
    v_all = v_pool.tile([128, NT, H * 65], BF16, name="v_all")
    for hh in range(H):
        nc.gpsimd.memset(v_all[:, :, 65 * hh + 64:65 * hh + 65], 1.0)

    h2T_pool = tc.alloc_tile_pool(name="h2Tp", bufs=1, side="right")
    h2T = h2T_pool.tile([128, NCH, T], BF16, name="h2T")
    outT_pool = tc.alloc_tile_pool(name="outTp", bufs=1, side="right")
    outT = outT_pool.tile([128, NPAIR, T], BF16, name="outT")

    wqk_tiles = {}

    def load_pair(pp):
        wq_t = w_pool.tile([128, C], BF16, tag="wq", name="wq_t")
        nc.sync.dma_start(out=wq_t, in_=wq_d[pp])
        wk_t = w_pool.tile([128, C], BF16, tag="wk", name="wk_t")
        nc.sync.dma_start(out=wk_t, in_=wk_d[pp])
        wqk_tiles[pp] = (wq_t, wk_t)

    # ---- Phase 1: LN1 -> hT (channel-major) + V ----
    v_view = v_all.rearrange("p i (h d) -> p i h d", h=H)
    with tc.tile_pool(name="h", bufs=3) as h_pool, \
         tc.tile_pool(name="xin1", bufs=4) as x_pool, \
         tc.tile_pool(name="wvg", bufs=1) as wv_pool:
        wv_t = wv_pool.tile([128, NCH * 1024], BF16, name="wv_t")

        def load_x(ii):
            x_t = x_pool.tile([128, C], F32, tag="x", name="x_t")
            for s in range(2):
                nc.sync.dma_start(
                    out=x_t[:, s * 512:(s + 1) * 512],
                    in_=x_d[ii * 128:(ii + 1) * 128, s * 512:(s + 1) * 512])
            return x_t

        x_ts = {0: load_x(0), 1: load_x(1)}
        for grp in range(2):
            nc.sync.dma_start(
                out=wv_t[:, grp * 4096:(grp + 1) * 4096],
                in_=wv_d[:, grp * 4096:(grp + 1) * 4096])
        load_pair(0)
        load_pair(1)
        stats = {0: _ln_stats(nc, ln_pool, x_ts[0], eps_tile)}
        for i in range(NT):
            if i + 2 < NT:
                x_ts[i + 2] = load_x(i + 2)
            rstd, negmr = stats.pop(i)
            x_t = x_ts.pop(i)
            h_t = h_pool.tile([128, C], BF16, tag="h", name="h_t")
            nc.scalar.activation(
                out=h_t, in_=x_t,
                func=mybir.ActivationFunctionType.Identity,
                bias=negmr, scale=rstd)
            ps_tr = ps_pool.tile([128, 1024], BF16, tag="sc01",
                                 bufs=2, name="ps_tr")
            for j in range(NCH):
                nc.tensor.transpose(
                    ps_tr[:, j * 128:(j + 1) * 128],
                    h_t[:, j * 128:(j + 1) * 128], ident)
            nc.vector.tensor_copy(
                out=hT[:, :, i * 128:(i + 1) * 128],
                in_=ps_tr.rearrange("p (j t) -> p j t", j=NCH))
            if i + 1 < NT:
                stats[i + 1] = _ln_stats(nc, ln_pool, x_ts[i + 1], eps_tile)
            for grp in range(2):
                ps_v = big()
                nc.tensor.matmul(
                    ps_v, ones_r, bv_sb[0:1, grp * 512:(grp + 1) * 512],
                    start=True, stop=False)
                for j in range(NCH):
                    nc.tensor.matmul(
                        ps_v, hT[:, j, i * 128:(i + 1) * 128],
                        wv_t[:, j * 1024 + grp * 512:j * 1024 + (grp + 1) * 512],
                        start=False, stop=(j == NCH - 1))
                nc.vector.tensor_copy(
                    out=v_view[:, i, grp * 8:(grp + 1) * 8, 0:64],
                    in_=ps_v.rearrange("p (h d) -> p h d", h=8))

    # hoist wo loads: DMA queue is idle during attention
    for ch in range(NCH):
        nc.sync.dma_start(
            out=wo_t[:, ch, :], in_=wo_d[ch * 128:(ch + 1) * 128, :])

    # ---- Phase 2: per head-pair QK + attention (V stationary) ----
    with tc.tile_pool(name="qk", bufs=2) as qk_pool, \
         tc.tile_pool(name="expS", bufs=20) as e_pool, \
         tc.tile_pool(name="rec", bufs=4) as r_pool, \
         tc.tile_pool(name="rbc", bufs=4) as rb_pool:
        for p in range(NPAIR):
            if p + 2 < NPAIR:
                load_pair(p + 2)
            wq_t, wk_t = wqk_tiles.pop(p)
            qT = qk_pool.tile([128, T], BF16, tag="qT", name="qT")
            kT = qk_pool.tile([128, T], BF16, tag="kT", name="kT")
            for dst, wt, bias in ((qT, wq_t, bq_sb), (kT, wk_t, bk_sb)):
                for half in range(2):
                    ps = big()
                    for j in range(NCH):
                        nc.tensor.matmul(
                            ps, wt[:, j * 128:(j + 1) * 128],
                            hT[:, j, half * 512:(half + 1) * 512],
                            start=(j == 0), stop=(j == NCH - 1))
                    nc.vector.tensor_scalar(
                        out=dst[:, half * 512:(half + 1) * 512], in0=ps,
                        scalar1=bias[:, p:p + 1], scalar2=None,
                        op0=mybir.AluOpType.add)

            for th in range(2):
                t0 = th * 512
                njt = (th + 1) * 4
                eS = [None] * njt
                for j in range(njt):
                    c0 = max(0, j * 128 - t0)
                    # both heads of the pair land in one 2-bank PSUM tile
                    ps = ps_pool.tile([128, 1024], F32, tag="sc01",
                                      bufs=2, name="ps_sc")
                    for hh in range(2):
                        hsl = slice(hh * 64, (hh + 1) * 64)
                        nc.tensor.matmul(
                            ps[:, hh * 512 + c0:(hh + 1) * 512],
                            kT[hsl, j * 128:(j + 1) * 128],
                            qT[hsl, t0 + c0:t0 + 512],
                            start=True, stop=True,
                            tile_position=(hh * 64, 0))
                    et = e_pool.tile([128, 1024], BF16, tag="e", name="eS_t")
                    pv = ps.rearrange("p (h q) -> p h q", h=2)
                    ev = et.rearrange("p (h q) -> p h q", h=2)
                    nc.scalar.activation(
                        out=ev[:, :, c0:512], in_=pv[:, :, c0:512],
                        func=mybir.ActivationFunctionType.Exp,
                        scale=float(HS) ** -0.5)
                    if j * 128 >= t0:
                        for hh in range(2):
                            nc.vector.memset(
                                et[64:128, hh * 512 + c0:hh * 512 + c0 + 64], 0.0)
                    eS[j] = et
                for hh in range(2):
                    head = 2 * p + hh
                    ps_av = ps_pool.tile([128, 512], F32, tag="av", bufs=2,
                                         name="ps_av")
                    for j in range(njt):
                        c0 = max(0, j * 128 - t0)
                        nc.tensor.matmul(
                            ps_av[0:65, c0:512],
                            v_all[:, j, 65 * head:65 * head + 65],
                            eS[j][:, hh * 512 + c0:(hh + 1) * 512],
                            start=(j == 0), stop=(j == njt - 1))
                    rc = r_pool.tile([128, 512], F32, tag="rc", name="rc")
                    _act_recip(nc, rc[0:1, :], ps_av[64:65, :])
                    rb = rb_pool.tile([128, 512], F32, tag="rb", name="rb")
                    nc.gpsimd.partition_broadcast(rb[0:64, :], rc[0:1, :])
                    nc.vector.tensor_tensor(
                        out=outT[64 * hh:64 * (hh + 1), p, t0:t0 + 512],
                        in0=ps_av[0:64, :], in1=rb[0:64, :],
                        op=mybir.AluOpType.mult)
    v_pool.release()
    w_pool.release()

    # hoist the first FFN1 weight tiles
    w1_tiles = {}

    def load_w1(gg):
        w1_t = w1_pool.tile([128, C], BF16, tag="w1", name="w1_t")
        nc.sync.dma_start(out=w1_t, in_=w1_d[gg])
        w1_tiles[gg] = w1_t

    for gg in range(3):
        load_w1(gg)

    # ---- Phase 3+4: projection + residual (+bo) + LN2 -> h2T ----
    # Skewed: transposes of tile i-1 are emitted after tile i's proj
    # matmuls so the PE never waits on the DVE/ACT LN2 chain.
    with tc.tile_pool(name="xin2", bufs=4) as x_pool:

        def load_x2(ii):
            x_t = x_pool.tile([128, C], F32, tag="x", name="x_t2")
            for s in range(2):
                nc.sync.dma_start(
                    out=x_t[:, s * 512:(s + 1) * 512],
                    in_=x_d[ii * 128:(ii + 1) * 128, s * 512:(s + 1) * 512])
            return x_t

        x_ts2 = {0: load_x2(0), 1: load_x2(1)}
        h_ts = {}

        def emit_tr2(ii):
            h_t = h_ts.pop(ii)
            ps_tr = ps_pool.tile([128, 1024], BF16, tag="sc01",
                                 bufs=2, name="ps_tr2")
            for j in range(NCH):
                nc.tensor.transpose(
                    ps_tr[:, j * 128:(j + 1) * 128],
                    h_t[:, j * 128:(j + 1) * 128], ident)
            nc.vector.tensor_copy(
                out=h2T[:, :, ii * 128:(ii + 1) * 128],
                in_=ps_tr.rearrange("p (j t) -> p j t", j=NCH))

        for i in range(NT):
            if i + 2 < NT:
                x_ts2[i + 2] = load_x2(i + 2)
            x_t = x_ts2.pop(i)
            for half in range(2):
                ps = big()
                nc.tensor.matmul(
                    ps, ones_r, bo_sb[0:1, half * 512:(half + 1) * 512],
                    start=True, stop=False)
                for ch in range(NCH):
                    nc.tensor.matmul(
                        ps, outT[:, ch, i * 128:(i + 1) * 128],
                        wo_t[:, ch, half * 512:(half + 1) * 512],
                        start=False, stop=(ch == NCH - 1))
                hsl = slice(half * 512, (half + 1) * 512)
                nc.vector.tensor_add(out=x2[:, i, hsl], in0=ps, in1=x_t[:, hsl])
            rstd, negmr = _ln_stats(nc, ln_pool, x2[:, i, :], eps_tile)
            h_t = x_pool.tile([128, C], BF16, tag="h2", name="h2_t")
            nc.scalar.activation(
                out=h_t, in_=x2[:, i, :],
                func=mybir.ActivationFunctionType.Identity,
                bias=negmr, scale=rstd)
            h_ts[i] = h_t
            if i >= 1:
                emit_tr2(i - 1)
        emit_tr2(NT - 1)
    outT_pool.release()
    wo_pool.release()

    # ---- Phase 5: FFN. W1 streamed once into full-T uT; W2 in 4 passes ----
    with tc.tile_pool(name="w2", bufs=8) as w2_pool, \
         tc.tile_pool(name="uTp", bufs=1) as uT_pool, \
         tc.tile_pool(name="yout", bufs=4) as out_pool:
        uT = uT_pool.tile([128, NG, T], BF16, name="uT")
        for g in range(NG):
            if g + 3 < NG:
                load_w1(g + 3)
            w1_t = w1_tiles.pop(g)
            for th in range(2):
                ps = big()
                for j in range(NCH):
                    nc.tensor.matmul(
                        ps, w1_t[:, j * 128:(j + 1) * 128],
                        h2T[:, j, th * 512:(th + 1) * 512],
                        start=(j == 0), stop=(j == NCH - 1))
                nc.vector.tensor_scalar(
                    out=uT[:, g, th * 512:(th + 1) * 512], in0=ps,
                    scalar1=b1_sb[:, g:g + 1], scalar2=0.0,
                    op0=mybir.AluOpType.add, op1=mybir.AluOpType.max)
        # FFN2: all 8 PSUM banks as full-T accumulators per column half,
        # so W2 streams exactly once (8MB bf16 total).
        for chh in range(2):
            hsl = slice(chh * 512, (chh + 1) * 512)
            ps_w = [ps_pool.tile([128, 1024], F32, tag="sc01",
                                 bufs=2, name=f"ps_w{iw}")
                    for iw in range(2)]
            ps_f = [ps_w[iw // 2][:, (iw % 2) * 512:(iw % 2 + 1) * 512]
                    for iw in range(4)]
            ps_f += [ps_pool.tile([128, 512], F32, tag="av", bufs=2,
                                  name=f"ps_a{iw}") for iw in range(2)]
            ps_f += [big(), big()]
            for it in range(8):
                nc.tensor.matmul(
                    ps_f[it], ones_r, b2_sb[0:1, hsl],
                    start=True, stop=False)
            for k in range(NG):
                w2_t = w2_pool.tile([128, 512], BF16, tag="w2", name="w2_t")
                nc.sync.dma_start(
                    out=w2_t, in_=w2_d[k * 128:(k + 1) * 128, hsl])
                for it in range(8):
                    nc.tensor.matmul(
                        ps_f[it],
                        uT[:, k, it * 128:(it + 1) * 128],
                        w2_t,
                        start=False, stop=(k == NG - 1))
            for it in range(8):
                o_t = out_pool.tile([128, 512], F32, tag="y", name="y_t")
                nc.vector.tensor_add(
                    out=o_t, in0=ps_f[it], in1=x2[:, it, hsl])
                nc.sync.dma_start(
                    out=y_d[it * 128:(it + 1) * 128, hsl], in_=o_t)
    h2T_pool.release()
    w1_pool.release()
    x2_pool.release()
    hT_pool.release()
    ps_pool.release()
    ln_pool.release()
    singles.release()


_NC_CACHE = {}


def _get_program():
    if "nc" not in _NC_CACHE:
        _NC_CACHE["nc"] = build_program()
    return _NC_CACHE["nc"]


def _prep_inputs(x, Wq, Wk, Wv, Wo, bo, ln1_g, ln1_b, ln2_g, ln2_b, W1, b1, W2, b2):
    import ml_dtypes
    BF = ml_dtypes.bfloat16
    f = lambda a: np.ascontiguousarray(np.asarray(a, dtype=np.float32))
    bf = lambda a: np.ascontiguousarray(np.asarray(a, np.float32).astype(BF))
    Wq, Wk, Wv = (np.asarray(w, np.float32) for w in (Wq, Wk, Wv))
    g1, b1l = np.asarray(ln1_g, np.float32), np.asarray(ln1_b, np.float32)
    g2, b2l = np.asarray(ln2_g, np.float32), np.asarray(ln2_b, np.float32)
    # [H,C,HS] -> [C, H*HS] with LN1 affine folded into the weights
    wq2 = Wq.transpose(1, 0, 2).reshape(C, C)
    wk2 = Wk.transpose(1, 0, 2).reshape(C, C)
    wv2 = Wv.transpose(1, 0, 2).reshape(C, C)
    bq, bk, bv = b1l @ wq2, b1l @ wk2, b1l @ wv2
    wq2, wk2, wv2 = g1[:, None] * wq2, g1[:, None] * wk2, g1[:, None] * wv2
    W1 = np.asarray(W1, np.float32)
    b1p = np.asarray(b1, np.float32) + b2l @ W1
    w1s = g2[:, None] * W1
    pack_p = lambda w: w.reshape(NCH, 128, NPAIR, 128).transpose(
        2, 1, 0, 3).reshape(NPAIR, 128, C)
    w1_pk = w1s.reshape(NCH, 128, NG, 128).transpose(2, 1, 0, 3).reshape(NG, 128, C)
    wv_pk = wv2.reshape(NCH, 128, C).transpose(1, 0, 2).reshape(128, NCH * C)
    return {
        "wq": bf(pack_p(wq2)), "wk": bf(pack_p(wk2)), "wv": bf(wv_pk),
        "wo": bf(Wo), "w1": bf(w1_pk), "w2": bf(W2),
        "bq": f(bq), "bk": f(bk), "bv": bf(bv),
        "bo": bf(bo), "b1": f(b1p), "b2": bf(b2),
    }


def kernel(x, mask, Wq, Wk, Wv, Wo, bo, ln1_g, ln1_b, ln2_g, ln2_b, W1, b1, W2, b2):
    x = np.ascontiguousarray(np.asarray(x, dtype=np.float32))
    B = x.shape[0]
    common = _prep_inputs(x, Wq, Wk, Wv, Wo, bo, ln1_g, ln1_b,
                          ln2_g, ln2_b, W1, b1, W2, b2)
    nc = _get_program()
    in_maps = [dict(common, x=np.ascontiguousarray(x[b])) for b in range(B)]
    res = run_bass_kernel_spmd(nc, in_maps, list(range(B)))
    return np.stack([res.results[b]["y"] for b in range(B)], axis=0)


# revision 21
# speedup vs baseline: 1096.4227x; 1.1585x over previous
"""Trainium2 Bass kernel for a dense transformer block.

Data-parallel over batch B=8 across 8 NeuronCores (one batch element per
core, weights replicated, no collectives).

Per core (x_b is [T=1024, C=1024] fp32):
  h  = LN1(x);  per-head q,k,v = h @ Wq/Wk/Wv;  S = q k^T / 8 with the
  "staircase" mask == block-causal at 64 granularity;  out = softmax(S) v
  x2 = x + cat(out) @ Wo + bo;  y = x2 + relu(LN2(x2) @ W1 + b1) @ W2 + b2

v3 design notes:
  - all matmul operands bf16 (weights pre-cast host-side); PSUM fp32.
  - LN affines folded into Wq/Wk/Wv/W1 host-side (h' = (x-m)*rstd only);
    the resulting per-channel biases enter as per-partition adds on the
    q/k copies, a K=1 ones-row matmul for v/bo/b2, and b1' in the relu.
  - attention A@V with V stationary ([keys,64+ones]), exp(S^T) moving:
    channel-major out, no output transposes; softmax denominator from
    the ones column, reciprocal on ScalarE (raw InstActivation),
    partition_broadcast on gpsimd, divide-multiply on DVE.
  - LN transposes in bf16, 8 per PSUM bank, single strided DVE copy out.
  - weights host-packed so every DMA is >=2KB/partition contiguous.
  - warm-up matmuls flip the PE HAM clock gate while x streams in.
"""

import os

import numpy as np

import concourse.bass as bass
import concourse.mybir as mybir
import concourse.tile as tile
from concourse import bacc
from concourse.masks import make_identity
from concourse.bass_utils import run_bass_kernel_spmd

T, C, H, HS = 1024, 1024, 16, 64
NT = T // 128
NCH = C // 128
NPAIR = H // 2
FF = 4 * C
NG = FF // 128
EPS = 1e-5
F32 = mybir.dt.float32
BF16 = mybir.dt.bfloat16


def _act_recip(nc, out, in_):
    """Reciprocal on ScalarE (accuracy ~1e-3, fine vs the 2e-2 gate)."""
    eng = nc.scalar
    ins = [eng.lower_ap(in_)]
    for v in (0.0, 1.0, 0.0):   # bias, scale, alpha
        ins.append(mybir.ImmediateValue(dtype=mybir.dt.float32, value=v))
    return eng.add_instruction(
        mybir.InstActivation(
            name=eng.bass.get_next_instruction_name(),
            func=mybir.ActivationFunctionType.Reciprocal,
            ins=ins, outs=[eng.lower_ap(out)]))


def _ln_stats(nc, pool, x_ap, eps_tile):
    """mean/rstd of [128,1024] fp32 tile -> (rstd, negmr) for ACT normalize."""
    stats = pool.tile([128, 2, 6], F32, tag="ln_stats", name="ln_stats")
    mv = pool.tile([128, 2], F32, tag="ln_mv", name="ln_mv")
    xr = x_ap.rearrange("p (s f) -> p s f", s=2)
    for s in range(2):
        nc.vector.bn_stats(out=stats[:, s, :], in_=xr[:, s, :])
    nc.vector.bn_aggr(out=mv, in_=stats)
    rstd = pool.tile([128, 1], F32, tag="ln_rstd", name="ln_rstd")
    nc.scalar.activation(
        out=rstd, in_=mv[:, 1:2],
        func=mybir.ActivationFunctionType.Sqrt,
        bias=eps_tile, scale=1.0,
    )
    nc.vector.reciprocal(out=rstd, in_=rstd)
    return mv, rstd


def build_program():
    nc = bacc.Bacc("TRN2", target_bir_lowering=False, debug=False, num_devices=8)

    x_d = nc.dram_tensor("x", [T, C], F32, kind="ExternalInput").ap()
    wq_d = nc.dram_tensor("wq", [NPAIR, 128, C], BF16, kind="ExternalInput").ap()
    wk_d = nc.dram_tensor("wk", [NPAIR, 128, C], BF16, kind="ExternalInput").ap()
    wv_d = nc.dram_tensor("wv", [128, NCH * C // 128 * 64 * 2], BF16,
                          kind="ExternalInput").ap()   # [128, 8192] packed
    wo_d = nc.dram_tensor("wo", [C, C], BF16, kind="ExternalInput").ap()
    w1_d = nc.dram_tensor("w1", [NG, 128, C], BF16, kind="ExternalInput").ap()
    w2_d = nc.dram_tensor("w2", [FF, C], BF16, kind="ExternalInput").ap()
    bq_d = nc.dram_tensor("bq", [C], F32, kind="ExternalInput").ap()
    bk_d = nc.dram_tensor("bk", [C], F32, kind="ExternalInput").ap()
    bv_d = nc.dram_tensor("bv", [C], BF16, kind="ExternalInput").ap()
    bo_d = nc.dram_tensor("bo", [C], BF16, kind="ExternalInput").ap()
    b1_d = nc.dram_tensor("b1", [FF], F32, kind="ExternalInput").ap()
    b2_d = nc.dram_tensor("b2", [C], BF16, kind="ExternalInput").ap()
    y_d = nc.dram_tensor("y", [T, C], F32, kind="ExternalOutput").ap()

    reps = int(os.environ.get("KERNEL_REPS", "1"))
    with tile.TileContext(nc) as tc:
        for r in range(reps):
            _emit(nc, tc, x_d, wq_d, wk_d, wv_d, wo_d, w1_d, w2_d,
                  bq_d, bk_d, bv_d, bo_d, b1_d, b2_d, y_d,
                  warmup=(r == 0))
    nc.compile()
    return nc


def _emit(nc, tc, x_d, wq_d, wk_d, wv_d, wo_d, w1_d, w2_d,
          bq_d, bk_d, bv_d, bo_d, b1_d, b2_d, y_d, warmup=False):
    singles = tc.alloc_tile_pool(name="singles", bufs=1)
    identf = singles.tile([128, 128], F32, name="identf")
    make_identity(nc, identf)
    ident = singles.tile([128, 128], BF16, name="ident")
    nc.vector.tensor_copy(out=ident, in_=identf)
    eps_tile = singles.tile([128, 1], F32, name="eps")
    nc.vector.memset(eps_tile, EPS)
    ones_r = singles.tile([1, 128], BF16, name="ones_r")
    nc.vector.memset(ones_r, 1.0)
    b1_sb = singles.tile([128, NG], F32, name="b1_sb")
    nc.sync.dma_start(out=b1_sb, in_=b1_d.rearrange("(g p) -> p g", p=128))
    bq_sb = singles.tile([128, NPAIR], F32, name="bq_sb")
    nc.sync.dma_start(out=bq_sb, in_=bq_d.rearrange("(g p) -> p g", p=128))
    bk_sb = singles.tile([128, NPAIR], F32, name="bk_sb")
    nc.sync.dma_start(out=bk_sb, in_=bk_d.rearrange("(g p) -> p g", p=128))
    bv_sb = singles.tile([1, C], BF16, name="bv_sb")
    nc.sync.dma_start(out=bv_sb, in_=bv_d.unsqueeze(0))
    bo_sb = singles.tile([1, C], BF16, name="bo_sb")
    nc.sync.dma_start(out=bo_sb, in_=bo_d.unsqueeze(0))
    b2_sb = singles.tile([1, C], BF16, name="b2_sb")
    nc.sync.dma_start(out=b2_sb, in_=b2_d.unsqueeze(0))

    ln_pool = tc.alloc_tile_pool(name="ln", bufs=3)

    # One global PSUM pool; every tile is one 2KB bank slot.
    ps_pool = tc.alloc_tile_pool(name="ps", bufs=1, space="PSUM")

    def big():
        return ps_pool.tile([128, 512], F32, tag="big", bufs=2, name="ps_big")

    if warmup:
        junk = singles.tile([128, 512], BF16, name="junk")
        nc.vector.memset(junk, 0.0)
        for _ in range(48):
            ps = ps_pool.tile([128, 512], F32, tag="av", bufs=2, name="ps_warm")
            nc.tensor.matmul(ps, ident, junk, start=True, stop=True)

    hT_pool = tc.alloc_tile_pool(name="hTp", bufs=1)
    hT = hT_pool.tile([128, NCH, T], BF16, name="hT")
    x2_pool = tc.alloc_tile_pool(name="x2p", bufs=1)
    x2 = x2_pool.tile([128, NT, C], F32, name="x2")
    w1_pool = tc.alloc_tile_pool(name="w1p", bufs=4)
    wo_pool = tc.alloc_tile_pool(name="wop", bufs=1)
    wo_t = wo_pool.tile([128, NCH, C], BF16, name="wo_t")
    w_pool = tc.alloc_tile_pool(name="wqk", bufs=2)
    v_pool = tc.alloc_tile_pool(name="vAp", bufs=1)
    v_all = v_pool.tile([128, NT, H * 65], BF16, name="v_all")
    for hh in range(H):
        nc.gpsimd.memset(v_all[:, :, 65 * hh + 64:65 * hh + 65], 1.0)

    h2T_pool = tc.alloc_tile_pool(name="h2Tp", bufs=1, side="right")
    h2T = h2T_pool.tile([128, NCH, T], BF16, name="h2T")
    outT_pool = tc.alloc_tile_pool(name="outTp", bufs=1, side="right")
    outT = outT_pool.tile([128, NPAIR, T], BF16, name="outT")

    wqk_tiles = {}

    def load_pair(pp):
        wq_t = w_pool.tile([128, C], BF16, tag="wq", name="wq_t")
        nc.sync.dma_start(out=wq_t, in_=wq_d[pp])
        wk_t = w_pool.tile([128, C], BF16, tag="wk", name="wk_t")
        nc.sync.dma_start(out=wk_t, in_=wk_d[pp])
        wqk_tiles[pp] = (wq_t, wk_t)

    # ---- Phase 1: LN1 -> hT (channel-major) + V ----
    v_view = v_all.rearrange("p i (h d) -> p i h d", h=H)
    with tc.tile_pool(name="h", bufs=3) as h_pool, \
         tc.tile_pool(name="xin1", bufs=4) as x_pool, \
         tc.tile_pool(name="wvg", bufs=1) as wv_pool:
        wv_t = wv_pool.tile([128, NCH * 1024], BF16, name="wv_t")

        def load_x(ii):
            x_t = x_pool.tile([128, C], F32, tag="x", name="x_t")
            for s in range(2):
                nc.sync.dma_start(
                    out=x_t[:, s * 512:(s + 1) * 512],
                    in_=x_d[ii * 128:(ii + 1) * 128, s * 512:(s + 1) * 512])
            return x_t

        x_ts = {0: load_x(0), 1: load_x(1)}
        for grp in range(2):
            nc.sync.dma_start(
                out=wv_t[:, grp * 4096:(grp + 1) * 4096],
                in_=wv_d[:, grp * 4096:(grp + 1) * 4096])
        load_pair(0)
        load_pair(1)
        stats = {0: _ln_stats(nc, ln_pool, x_ts[0], eps_tile)}
        for i in range(NT):
            if i + 2 < NT:
                x_ts[i + 2] = load_x(i + 2)
            mv, rstd = stats.pop(i)
            x_t = x_ts.pop(i)
            h_t = h_pool.tile([128, C], BF16, tag="h", name="h_t")
            nc.vector.tensor_scalar(
                out=h_t, in0=x_t,
                scalar1=mv[:, 0:1], scalar2=rstd,
                op0=mybir.AluOpType.subtract, op1=mybir.AluOpType.mult)
            ps_tr = ps_pool.tile([128, 1024], BF16, tag="sc01",
                                 bufs=2, name="ps_tr")
            for j in range(NCH):
                nc.tensor.transpose(
                    ps_tr[:, j * 128:(j + 1) * 128],
                    h_t[:, j * 128:(j + 1) * 128], ident)
            nc.scalar.activation(
                out=hT[:, :, i * 128:(i + 1) * 128],
                in_=ps_tr.rearrange("p (j t) -> p j t", j=NCH),
                func=mybir.ActivationFunctionType.Copy)
            if i + 1 < NT:
                stats[i + 1] = _ln_stats(nc, ln_pool, x_ts[i + 1], eps_tile)
            for grp in range(2):
                ps_v = big()
                nc.tensor.matmul(
                    ps_v, ones_r, bv_sb[0:1, grp * 512:(grp + 1) * 512],
                    start=True, stop=False)
                for j in range(NCH):
                    nc.tensor.matmul(
                        ps_v, hT[:, j, i * 128:(i + 1) * 128],
                        wv_t[:, j * 1024 + grp * 512:j * 1024 + (grp + 1) * 512],
                        start=False, stop=(j == NCH - 1))
                nc.vector.tensor_copy(
                    out=v_view[:, i, grp * 8:(grp + 1) * 8, 0:64],
                    in_=ps_v.rearrange("p (h d) -> p h d", h=8))

    # hoist wo loads: DMA queue is idle during attention
    for ch in range(NCH):
        nc.sync.dma_start(
            out=wo_t[:, ch, :], in_=wo_d[ch * 128:(ch + 1) * 128, :])

    # ---- Phase 2: per head-pair QK + attention (V stationary) ----
    with tc.tile_pool(name="qk", bufs=2) as qk_pool, \
         tc.tile_pool(name="expS", bufs=20) as e_pool, \
         tc.tile_pool(name="rec", bufs=4) as r_pool, \
         tc.tile_pool(name="rbc", bufs=4) as rb_pool:
        for p in range(NPAIR):
            if p + 2 < NPAIR:
                load_pair(p + 2)
            wq_t, wk_t = wqk_tiles.pop(p)
            qT = qk_pool.tile([128, T], BF16, tag="qT", name="qT")
            kT = qk_pool.tile([128, T], BF16, tag="kT", name="kT")
            for dst, wt, bias in ((qT, wq_t, bq_sb), (kT, wk_t, bk_sb)):
                for half in range(2):
                    ps = big()
                    for j in range(NCH):
                        nc.tensor.matmul(
                            ps, wt[:, j * 128:(j + 1) * 128],
                            hT[:, j, half * 512:(half + 1) * 512],
                            start=(j == 0), stop=(j == NCH - 1))
                    nc.vector.tensor_scalar(
                        out=dst[:, half * 512:(half + 1) * 512], in0=ps,
                        scalar1=bias[:, p:p + 1], scalar2=None,
                        op0=mybir.AluOpType.add)

            for th in range(2):
                t0 = th * 512
                njt = (th + 1) * 4
                eS = [None] * njt
                for j in range(njt):
                    c0 = max(0, j * 128 - t0)
                    # both heads of the pair land in one 2-bank PSUM tile
                    ps = ps_pool.tile([128, 1024], F32, tag="sc01",
                                      bufs=2, name="ps_sc")
                    for hh in range(2):
                        hsl = slice(hh * 64, (hh + 1) * 64)
                        nc.tensor.matmul(
                            ps[:, hh * 512 + c0:(hh + 1) * 512],
                            kT[hsl, j * 128:(j + 1) * 128],
                            qT[hsl, t0 + c0:t0 + 512],
                            start=True, stop=True,
                            tile_position=(hh * 64, 0))
                    et = e_pool.tile([128, 1024], BF16, tag="e", name="eS_t")
                    pv = ps.rearrange("p (h q) -> p h q", h=2)
                    ev = et.rearrange("p (h q) -> p h q", h=2)
                    nc.scalar.activation(
                        out=ev[:, :, c0:512], in_=pv[:, :, c0:512],
                        func=mybir.ActivationFunctionType.Exp,
                        scale=float(HS) ** -0.5)
                    if j * 128 >= t0:
                        for hh in range(2):
                            nc.vector.memset(
                                et[64:128, hh * 512 + c0:hh * 512 + c0 + 64], 0.0)
                    eS[j] = et
                for hh in range(2):
                    head = 2 * p + hh
                    ps_av = ps_pool.tile([128, 512], F32, tag="av", bufs=2,
                                         name="ps_av")
                    for j in range(njt):
                        c0 = max(0, j * 128 - t0)
                        nc.tensor.matmul(
                            ps_av[0:65, c0:512],
                            v_all[:, j, 65 * head:65 * head + 65],
                            eS[j][:, hh * 512 + c0:(hh + 1) * 512],
                            start=(j == 0), stop=(j == njt - 1))
                    rc = r_pool.tile([128, 512], F32, tag="rc", name="rc")
                    _act_recip(nc, rc[0:1, :], ps_av[64:65, :])
                    rb = rb_pool.tile([128, 512], F32, tag="rb", name="rb")
                    nc.gpsimd.partition_broadcast(rb[0:64, :], rc[0:1, :])
                    nc.vector.tensor_tensor(
                        out=outT[64 * hh:64 * (hh + 1), p, t0:t0 + 512],
                        in0=ps_av[0:64, :], in1=rb[0:64, :],
                        op=mybir.AluOpType.mult)
    v_pool.release()
    w_pool.release()

    # hoist the first FFN1 weight tiles
    w1_tiles = {}

    def load_w1(gg):
        w1_t = w1_pool.tile([128, C], BF16, tag="w1", name="w1_t")
        nc.sync.dma_start(out=w1_t, in_=w1_d[gg])
        w1_tiles[gg] = w1_t

    for gg in range(3):
        load_w1(gg)

    # ---- Phase 3+4: projection + residual (+bo) + LN2 -> h2T ----
    # Skewed: transposes of tile i-1 are emitted after tile i's proj
    # matmuls so the PE never waits on the DVE/ACT LN2 chain.
    with tc.tile_pool(name="xin2", bufs=4) as x_pool:

        def load_x2(ii):
            x_t = x_pool.tile([128, C], F32, tag="x", name="x_t2")
            for s in range(2):
                nc.sync.dma_start(
                    out=x_t[:, s * 512:(s + 1) * 512],
                    in_=x_d[ii * 128:(ii + 1) * 128, s * 512:(s + 1) * 512])
            return x_t

        x_ts2 = {0: load_x2(0), 1: load_x2(1)}
        h_ts = {}

        def emit_tr2(ii):
            h_t = h_ts.pop(ii)
            ps_tr = ps_pool.tile([128, 1024], BF16, tag="sc01",
                                 bufs=2, name="ps_tr2")
            for j in range(NCH):
                nc.tensor.transpose(
                    ps_tr[:, j * 128:(j + 1) * 128],
                    h_t[:, j * 128:(j + 1) * 128], ident)
            nc.scalar.activation(
                out=h2T[:, :, ii * 128:(ii + 1) * 128],
                in_=ps_tr.rearrange("p (j t) -> p j t", j=NCH),
                func=mybir.ActivationFunctionType.Copy)

        for i in range(NT):
            if i + 2 < NT:
                x_ts2[i + 2] = load_x2(i + 2)
            x_t = x_ts2.pop(i)
            ps = ps_pool.tile([128, 1024], F32, tag="sc01", bufs=2,
                              name="ps_pr")
            for half in range(2):
                psh = ps[:, half * 512:(half + 1) * 512]
                nc.tensor.matmul(
                    psh, ones_r, bo_sb[0:1, half * 512:(half + 1) * 512],
                    start=True, stop=False)
                for ch in range(NCH):
                    nc.tensor.matmul(
                        psh, outT[:, ch, i * 128:(i + 1) * 128],
                        wo_t[:, ch, half * 512:(half + 1) * 512],
                        start=False, stop=(ch == NCH - 1))
            nc.vector.tensor_add(out=x2[:, i, :], in0=ps, in1=x_t)
            mv, rstd = _ln_stats(nc, ln_pool, x2[:, i, :], eps_tile)
            h_t = x_pool.tile([128, C], BF16, tag="h2", name="h2_t")
            nc.vector.tensor_scalar(
                out=h_t, in0=x2[:, i, :],
                scalar1=mv[:, 0:1], scalar2=rstd,
                op0=mybir.AluOpType.subtract, op1=mybir.AluOpType.mult)
            h_ts[i] = h_t
            if i >= 1:
                emit_tr2(i - 1)
        emit_tr2(NT - 1)
    outT_pool.release()
    wo_pool.release()

    # ---- Phase 5: FFN. W1 streamed once into full-T uT; W2 in 4 passes ----
    with tc.tile_pool(name="w2", bufs=8) as w2_pool, \
         tc.tile_pool(name="uTp", bufs=1) as uT_pool, \
         tc.tile_pool(name="yout", bufs=4) as out_pool:
        uT = uT_pool.tile([128, NG, T], BF16, name="uT")
        for g in range(NG):
            if g + 3 < NG:
                load_w1(g + 3)
            w1_t = w1_tiles.pop(g)
            for th in range(2):
                ps = big()
                for j in range(NCH):
                    nc.tensor.matmul(
                        ps, w1_t[:, j * 128:(j + 1) * 128],
                        h2T[:, j, th * 512:(th + 1) * 512],
                        start=(j == 0), stop=(j == NCH - 1))
                nc.vector.tensor_scalar(
                    out=uT[:, g, th * 512:(th + 1) * 512], in0=ps,
                    scalar1=b1_sb[:, g:g + 1], scalar2=0.0,
                    op0=mybir.AluOpType.add, op1=mybir.AluOpType.max)
        # FFN2: all 8 PSUM banks as full-T accumulators per column half,
        # so W2 streams exactly once (8MB bf16 total).
        for chh in range(2):
            hsl = slice(chh * 512, (chh + 1) * 512)
            ps_w = [ps_pool.tile([128, 1024], F32, tag="sc01",
                                 bufs=2, name=f"ps_w{iw}")
                    for iw in range(2)]
            ps_f = [ps_w[iw // 2][:, (iw % 2) * 512:(iw % 2 + 1) * 512]
                    for iw in range(4)]
            ps_f += [ps_pool.tile([128, 512], F32, tag="av", bufs=2,
                                  name=f"ps_a{iw}") for iw in range(2)]
            ps_f += [big(), big()]
            for it in range(8):
                nc.tensor.matmul(
                    ps_f[it], ones_r, b2_sb[0:1, hsl],
                    start=True, stop=False)
            for k in range(NG):
                w2_t = w2_pool.tile([128, 512], BF16, tag="w2", name="w2_t")
                nc.sync.dma_start(
                    out=w2_t, in_=w2_d[k * 128:(k + 1) * 128, hsl])
                for it in range(8):
                    nc.tensor.matmul(
                        ps_f[it],
                        uT[:, k, it * 128:(it + 1) * 128],
                        w2_t,
                        start=False, stop=(k == NG - 1))
            for it in range(8):
                o_t = out_pool.tile([128, 512], F32, tag="y", name="y_t")
                nc.vector.tensor_add(
                    out=o_t, in0=ps_f[it], in1=x2[:, it, hsl])
                nc.sync.dma_start(
                    out=y_d[it * 128:(it + 1) * 128, hsl], in_=o_t)
    h2T_pool.release()
    w1_pool.release()
    x2_pool.release()
    hT_pool.release()
    ps_pool.release()
    ln_pool.release()
    singles.release()


_NC_CACHE = {}


def _get_program():
    if "nc" not in _NC_CACHE:
        _NC_CACHE["nc"] = build_program()
    return _NC_CACHE["nc"]


def _prep_inputs(x, Wq, Wk, Wv, Wo, bo, ln1_g, ln1_b, ln2_g, ln2_b, W1, b1, W2, b2):
    import ml_dtypes
    BF = ml_dtypes.bfloat16
    f = lambda a: np.ascontiguousarray(np.asarray(a, dtype=np.float32))
    bf = lambda a: np.ascontiguousarray(np.asarray(a, np.float32).astype(BF))
    Wq, Wk, Wv = (np.asarray(w, np.float32) for w in (Wq, Wk, Wv))
    g1, b1l = np.asarray(ln1_g, np.float32), np.asarray(ln1_b, np.float32)
    g2, b2l = np.asarray(ln2_g, np.float32), np.asarray(ln2_b, np.float32)
    # [H,C,HS] -> [C, H*HS] with LN1 affine folded into the weights
    wq2 = Wq.transpose(1, 0, 2).reshape(C, C)
    wk2 = Wk.transpose(1, 0, 2).reshape(C, C)
    wv2 = Wv.transpose(1, 0, 2).reshape(C, C)
    bq, bk, bv = b1l @ wq2, b1l @ wk2, b1l @ wv2
    wq2, wk2, wv2 = g1[:, None] * wq2, g1[:, None] * wk2, g1[:, None] * wv2
    W1 = np.asarray(W1, np.float32)
    b1p = np.asarray(b1, np.float32) + b2l @ W1
    w1s = g2[:, None] * W1
    pack_p = lambda w: w.reshape(NCH, 128, NPAIR, 128).transpose(
        2, 1, 0, 3).reshape(NPAIR, 128, C)
    w1_pk = w1s.reshape(NCH, 128, NG, 128).transpose(2, 1, 0, 3).reshape(NG, 128, C)
    wv_pk = wv2.reshape(NCH, 128, C).transpose(1, 0, 2).reshape(128, NCH * C)
    return {
        "wq": bf(pack_p(wq2)), "wk": bf(pack_p(wk2)), "wv": bf(wv_pk),
        "wo": bf(Wo), "w1": bf(w1_pk), "w2": bf(W2),
        "bq": f(bq), "bk": f(bk), "bv": bf(bv),
        "bo": bf(bo), "b1": f(b1p), "b2": bf(b2),
    }


def kernel(x, mask, Wq, Wk, Wv, Wo, bo, ln1_g, ln1_b, ln2_g, ln2_b, W1, b1, W2, b2):
    x = np.ascontiguousarray(np.asarray(x, dtype=np.float32))
    B = x.shape[0]
    common = _prep_inputs(x, Wq, Wk, Wv, Wo, bo, ln1_g, ln1_b,
                          ln2_g, ln2_b, W1, b1, W2, b2)
    nc = _get_program()
    in_maps = [dict(common, x=np.ascontiguousarray(x[b])) for b in range(B)]
    res = run_bass_kernel_spmd(nc, in_maps, list(range(B)))
    return np.stack([res.results[b]["y"] for b in range(B)], axis=0)
